# revision 42
# baseline (speedup 1.0000x reference)
import os
import sys

sys.path.insert(0, "/opt/trn_rl_repo")

_DBG = int(os.environ.get("KDBG", "9"))  # phase cutoff for fault bisection

from contextlib import ExitStack

import numpy as np
import ml_dtypes
import concourse.bacc as bacc
import concourse.bass as bass
import concourse.mybir as mybir
from concourse.bass_utils import run_bass_kernel_spmd
from concourse.tile import TileContext
from concourse.library_config import mlp as _mlp_lib
from concourse.masks import make_identity

P = 128
NCORES = 8
N, D, E, KHOP, B, L = 100000, 128, 1600000, 3, 32768, 262144
H_MLP, R = 512, 64
NBLK = 98              # local node blocks per core
SHARD = NBLK * P       # 12544 rows per core
NP = NCORES * SHARD    # 100352 padded rows
NBLK_G = NCORES * NBLK # 784 global dst blocks
NSB_G = B // P         # 256 global segment blocks
BSEG = B // NCORES     # 4096 segments per core
NSB = BSEG // P        # 32 local segment blocks
GCALL = 2048           # tokens per hop dma_gather call
SPC = GCALL // P       # slabs per hop gather call
PCALL = 2048           # tokens per pool dma_gather call (512B rows)
SPCP = PCALL // P      # slabs per pool gather call

f32 = mybir.dt.float32
bf16 = mybir.dt.bfloat16
i16 = mybir.dt.int16
i32 = mybir.dt.int32

_COMPILED = {}


def _wrap_idx16(idx):
    """dma_gather index layout: token i -> partition i%16, col i//16,
    replicated x8 to fill 128 partitions."""
    n = len(idx)
    assert n % 16 == 0
    return np.tile(idx.reshape(n // 16, 16).T.astype(np.int16), (8, 1))


def _grow(local):
    """Gather row of a local node index in the [p, l] table layout."""
    return (local & 127) * NBLK + (local >> 7)


def _schedule(per_core, nb, call):
    """per_core: list of (blk, loc, gidx) arrays, sorted by blk.
    Returns a shared straddled schedule plus per-core token/loc data."""
    cnts = np.zeros((NCORES, nb), np.int64)
    for c, (blk, _, _) in enumerate(per_core):
        cnts[c] = np.bincount(blk, minlength=nb)
    gsz = np.maximum(cnts.max(axis=0), 1)
    off = np.zeros(nb + 1, np.int64)
    off[1:] = np.cumsum(gsz)
    total = int(off[nb])
    tot_pad = (total + call - 1) // call * call
    mm = []  # (block, slab) in consumption order
    for t in range(nb):
        s0, s1 = off[t] // P, (off[t] + gsz[t] - 1) // P
        for s in range(s0, s1 + 1):
            mm.append((t, int(s)))
    M = len(mm)
    gstreams, loccols = [], []
    for c, (blk, loc, gidx) in enumerate(per_core):
        starts = np.concatenate([[0], np.cumsum(cnts[c])])
        pos = off[blk] + (np.arange(len(blk)) - starts[blk])
        g = np.zeros(tot_pad, np.int16)
        g[pos] = gidx
        lstream = np.full(tot_pad, -1.0, np.float32)
        lstream[pos] = loc
        cols = np.full((M, P), -1.0, np.float32)
        ar = np.arange(P)
        for m, (t, s) in enumerate(mm):
            idxs = s * P + ar
            seg = lstream[s * P:(s + 1) * P]
            mask = (idxs >= off[t]) & (idxs < off[t] + gsz[t])
            cols[m] = np.where(mask, seg, -1.0)
        gstreams.append(g)
        loccols.append(np.ascontiguousarray(cols.T))
    return tuple(int(v) for v in gsz), tot_pad, mm, gstreams, loccols


def _prep_hop(src, dst):
    per_core = []
    for c in range(NCORES):
        m = (src >= c * SHARD) & (src < (c + 1) * SHARD)
        sl = src[m] - c * SHARD
        dg = dst[m]
        t = dg >> 7
        order = np.argsort(t, kind="stable")
        t, sl, dg = t[order], sl[order], dg[order]
        per_core.append((t, (dg & 127).astype(np.float32),
                         _grow(sl).astype(np.int16)))
    return _schedule(per_core, NBLK_G, GCALL)


def _prep_pool(idx, seg):
    per_core = []
    for c in range(NCORES):
        m = (idx >= c * SHARD) & (idx < (c + 1) * SHARD)
        r = idx[m] - c * SHARD
        sg = seg[m]
        sb = sg >> 7
        order = np.argsort(sb, kind="stable")
        sb, r, sg = sb[order], r[order], sg[order]
        per_core.append((sb, (sg & 127).astype(np.float32),
                         _grow(r).astype(np.int16)))
    return _schedule(per_core, NSB_G, PCALL)


def _build_program(hsch, psch_h, psch_t):
    nc = bacc.Bacc("TRN2", target_bir_lowering=False, num_devices=NCORES)

    gsz_h, tok_h, mm_h, _, _ = hsch
    gsz_ph, tok_ph, mm_ph, _, _ = psch_h
    gsz_pt, tok_pt, mm_pt, _, _ = psch_t
    off_h = np.zeros(NBLK_G + 1, np.int64)
    off_h[1:] = np.cumsum(gsz_h)

    embed_in = nc.dram_tensor("embed_in", [P, NBLK, D], f32, kind="ExternalInput")
    temp_in = nc.dram_tensor("temp_in", [P, 4], f32, kind="ExternalInput")
    a_in = nc.dram_tensor("a_in", [P, NBLK], f32, kind="ExternalInput")
    ab_in = nc.dram_tensor("ab_in", [P, NBLK], f32, kind="ExternalInput")
    bt_in = nc.dram_tensor("bt_in", [P, 3, NBLK], f32, kind="ExternalInput")
    wrep_in = nc.dram_tensor("wrep_in", [P, D], f32, kind="ExternalInput")
    w1_in = nc.dram_tensor("w1_in", [3, P, H_MLP], bf16, kind="ExternalInput")
    b1_in = nc.dram_tensor("b1_in", [P, 4], f32, kind="ExternalInput")
    w2_in = nc.dram_tensor("w2_in", [4, P, R], bf16, kind="ExternalInput")
    b2_in = nc.dram_tensor("b2_in", [R, 1], f32, kind="ExternalInput")
    hsrc = nc.dram_tensor("hsrc", [P, tok_h // 16], i16, kind="ExternalInput")
    hloc = nc.dram_tensor("hloc", [P, len(mm_h)], f32, kind="ExternalInput")
    psrcH = nc.dram_tensor("psrcH", [P, tok_ph // 16], i16, kind="ExternalInput")
    psrcT = nc.dram_tensor("psrcT", [P, tok_pt // 16], i16, kind="ExternalInput")
    plocH = nc.dram_tensor("plocH", [P, len(mm_ph)], f32, kind="ExternalInput")
    plocT = nc.dram_tensor("plocT", [P, len(mm_pt)], f32, kind="ExternalInput")

    out = nc.dram_tensor("out", [BSEG, R], f32, kind="ExternalOutput")

    xs_a = nc.dram_tensor("xs_a", [SHARD, D], bf16)
    xs_b = nc.dram_tensor("xs_b", [SHARD, D], bf16)
    part_a = nc.dram_tensor("part_a", [NP, D], bf16)
    part_b = nc.dram_tensor("part_b", [NP, D], bf16)
    rsx = nc.dram_tensor("rsx", [SHARD, D], bf16)
    zs = nc.dram_tensor("zs", [SHARD, 2 * D], bf16)
    pool_part = nc.dram_tensor("pool_part", [2 * B, 2 * D], bf16)
    rs_p = nc.dram_tensor("rs_p", [2 * BSEG, 2 * D], bf16)

    rg = [list(range(NCORES))]

    def xview(t):  # [p, l, d] view of a [SHARD, D] table
        return t.rearrange("(p l) d -> p l d", p=P)

    part_av = part_a.rearrange("(o p l) d -> o p l d", o=NCORES, p=P)
    part_bv = part_b.rearrange("(o p l) d -> o p l d", o=NCORES, p=P)
    ppv = pool_part.rearrange("(t o s l) d -> t o s l d", t=2, o=NCORES, s=P)
    rspv = rs_p.rearrange("(t s l) d -> t s l d", t=2, s=P)

    with TileContext(nc) as tc, ExitStack() as ctx:
        sb = ctx.enter_context(tc.tile_pool(name="sb", bufs=3))
        const = ctx.enter_context(tc.tile_pool(name="const", bufs=1))
        ohp = ctx.enter_context(tc.tile_pool(name="ohp", bufs=6))
        ccs = ctx.enter_context(nc.semaphore("ccs"))
        ccs_val = [0]

        def rscatter(ins_ap, outs_ap):
            tc.strict_bb_all_engine_barrier()
            with tc.tile_critical():
                ccs_val[0] += 1
                nc.gpsimd.collective_compute(
                    "ReduceScatter", mybir.AluOpType.add,
                    ins=[ins_ap], outs=[outs_ap], replica_groups=rg,
                ).then_inc(ccs, 1)
                nc.gpsimd.wait_ge(ccs, ccs_val[0])
            tc.strict_bb_all_engine_barrier()

        nc.gpsimd.load_library(_mlp_lib)

        # ---------- constants ----------
        iota_i = const.tile([P, P], i32)
        nc.gpsimd.iota(iota_i[:], pattern=[[1, P]], base=0, channel_multiplier=0)
        iota_b = const.tile([P, P], bf16)
        nc.vector.tensor_copy(iota_b[:], iota_i[:])
        ident = const.tile([P, P], f32)
        make_identity(nc, ident[:])

        temp_sb = const.tile([P, 4], f32)
        nc.sync.dma_start(temp_sb[:], temp_in[:])
        a_sc = const.tile([P, NBLK], f32)
        nc.sync.dma_start(a_sc[:], a_in[:])
        ab_sc = const.tile([P, NBLK], f32)
        nc.sync.dma_start(ab_sc[:], ab_in[:])
        bt_sc = const.tile([P, 3, NBLK], f32)
        nc.sync.dma_start(bt_sc[:], bt_in[:])
        wrep = const.tile([P, D], f32)
        nc.sync.dma_start(wrep[:], wrep_in[:])
        w1t = const.tile([P, 3, H_MLP], bf16)
        nc.sync.dma_start(w1t[:], w1_in.rearrange("k p h -> p k h")[:])
        b1t = const.tile([P, 4], f32)
        nc.sync.dma_start(b1t[:], b1_in[:])
        w2t = const.tile([P, 4, R], bf16)
        nc.sync.dma_start(w2t[:], w2_in.rearrange("k p r -> p k r")[:])
        b2t = const.tile([R, 1], f32)
        nc.sync.dma_start(b2t[:], b2_in[:])

        hop_idx = const.tile([P, tok_h // 16], i16)
        nc.sync.dma_start(hop_idx[:], hsrc[:])
        hop_loc = const.tile([P, len(mm_h)], f32)
        nc.sync.dma_start(hop_loc[:], hloc[:])
        pool_idx_h = const.tile([P, tok_ph // 16], i16)
        nc.sync.dma_start(pool_idx_h[:], psrcH[:])
        pool_idx_t = const.tile([P, tok_pt // 16], i16)
        nc.sync.dma_start(pool_idx_t[:], psrcT[:])
        pool_loc_h = const.tile([P, len(mm_ph)], f32)
        nc.sync.dma_start(pool_loc_h[:], plocH[:])
        pool_loc_t = const.tile([P, len(mm_pt)], f32)
        nc.sync.dma_start(pool_loc_t[:], plocT[:])

        # ---------- init: hidden = temp0*embed, xs_a = a_sc*embed ----------
        stp = ctx.enter_context(tc.tile_pool(name="stp", bufs=3))
        hidden = const.tile([P, NBLK, D], f32)
        nc.sync.dma_start(hidden[:], embed_in[:])
        for l0 in range(0, NBLK, 16):
            l1 = min(l0 + 16, NBLK)
            stg = stp.tile([P, 16, D], bf16, tag="xst")
            for l in range(l0, l1):
                nc.scalar.activation(stg[:, l - l0, :], hidden[:, l, :],
                                     mybir.ActivationFunctionType.Copy,
                                     scale=a_sc[:, l:l + 1])
            nc.sync.dma_start(xview(xs_a)[:, l0:l1, :], stg[:, :l1 - l0, :])
        nc.vector.tensor_scalar(out=hidden[:], in0=hidden[:],
                                scalar1=temp_sb[:, 0:1], scalar2=None,
                                op0=mybir.AluOpType.mult)

        # ---------- hops ----------
        # precompute per-block slab spans
        spans = []
        for t in range(NBLK_G):
            s0 = int(off_h[t]) // P
            s1 = int(off_h[t] + gsz_h[t] - 1) // P
            spans.append((s0, s1))

        with tc.tile_pool(name="psh", bufs=6, space="PSUM") as psh, \
                tc.tile_pool(name="ghop", bufs=3) as gpool:
            for k in range(KHOP if _DBG >= 4 else (1 if _DBG >= 2 else 0)):
                xs_src = xs_a if k % 2 == 0 else xs_b
                xs_dst = xs_b if k % 2 == 0 else xs_a
                part = part_a if k % 2 == 0 else part_b
                partv = part_av if k % 2 == 0 else part_bv

                # xs_src DRAM writes (init / previous post-phase) must land
                # before this hop's gathers read them (DRAM is untracked).
                tc.strict_bb_all_engine_barrier()

                tiles = {}
                hi_call = [-1]

                def need_call(s, xs_src=xs_src, tiles=tiles, hi_call=hi_call,
                              toktot=tok_h, idx_t=hop_idx, tabsrc=None, w=D):
                    ci = s // SPC
                    while hi_call[0] < ci:
                        cj = hi_call[0] + 1
                        n_ = min(GCALL, toktot - cj * GCALL)
                        gt = gpool.tile([P, n_ // P, w], bf16, tag="gt")
                        nc.gpsimd.dma_gather(
                            gt[:], xs_src[:],
                            idx_t[:, cj * GCALL // 16:(cj * GCALL + n_) // 16],
                            n_, n_, w, single_packet=False)
                        tiles[cj] = gt
                        tiles.pop(cj - 4, None)
                        hi_call[0] = cj
                    return tiles[ci]

                mcol = 0
                stg = None
                for t in range(NBLK_G):
                    o, l = t // NBLK, t % NBLK
                    s0, s1 = spans[t]
                    acc = psh.tile([P, D], f32, tag="acc")
                    for s in range(s0, s1 + 1):
                        gt = need_call(s)
                        oh = ohp.tile([P, P], bf16, tag="oh")
                        eng = nc.gpsimd if mcol % 3 == 2 else nc.vector
                        eng.tensor_scalar(
                            out=oh[:], in0=iota_b[:],
                            scalar1=hop_loc[:, mcol:mcol + 1],
                            scalar2=None, op0=mybir.AluOpType.is_equal)
                        nc.tensor.matmul(acc[:], lhsT=oh[:],
                                         rhs=gt[:, s % SPC, :],
                                         start=s == s0, stop=s == s1)
                        mcol += 1
                    if l % 16 == 0:
                        stg = stp.tile([P, 16, D], bf16, tag="fst")
                    nc.any.tensor_copy(stg[:, l % 16, :], acc[:])
                    if l % 16 == 15 or l == NBLK - 1:
                        lb = l // 16 * 16
                        nc.sync.dma_start(partv[o, :, lb:l + 1, :],
                                          stg[:, :l - lb + 1, :])
                assert mcol == len(mm_h)

                # reduce-scatter partial -> own shard; update hidden/xs.
                # xs_dst writes are emitted first (they gate the next hop's
                # gathers via the top-of-hop barrier); the hidden updates are
                # appended after and overlap the next hop's compute.
                if _DBG < 3:
                    continue
                rscatter(part[:], rsx[:])
                for l0 in range(0, NBLK, 16):
                    l1 = min(l0 + 16, NBLK)
                    rsb = sb.tile([P, 16, D], bf16, tag="rsb")
                    nc.sync.dma_start(rsb[:, :l1 - l0, :],
                                      xview(rsx)[:, l0:l1, :])
                    if k < KHOP - 1:
                        stg2 = stp.tile([P, 16, D], bf16, tag="xst")
                    for l in range(l0, l1):
                        tmp = sb.tile([P, D], f32, tag="tmp")
                        nc.scalar.activation(tmp[:], rsb[:, l - l0, :],
                                             mybir.ActivationFunctionType.Copy,
                                             scale=bt_sc[:, k, l:l + 1])
                        nc.any.tensor_tensor(out=hidden[:, l, :],
                                             in0=hidden[:, l, :], in1=tmp[:],
                                             op=mybir.AluOpType.add)
                        if k < KHOP - 1:
                            nc.scalar.activation(
                                stg2[:, l - l0, :], rsb[:, l - l0, :],
                                mybir.ActivationFunctionType.Copy,
                                scale=ab_sc[:, l:l + 1])
                    if k < KHOP - 1:
                        nc.sync.dma_start(xview(xs_dst)[:, l0:l1, :],
                                          stg2[:, :l1 - l0, :])

        # ---------- z_ext = [z*e | e | junk] to zs ----------
        zsv = zs.rearrange("(p l) d -> p l d", p=P)
        for l0 in range(0 if _DBG >= 5 else NBLK, NBLK, 8):
            l1 = min(l0 + 8, NBLK)
            zst = stp.tile([P, 8, 2 * D], bf16, tag="zst")
            for l in range(l0, l1):
                prod = sb.tile([P, D], f32, tag="prod")
                nc.any.tensor_tensor(out=prod[:], in0=hidden[:, l, :],
                                     in1=wrep[:], op=mybir.AluOpType.mult)
                scol = sb.tile([P, 1], f32, tag="scol")
                nc.vector.reduce_sum(scol[:], prod[:], axis=mybir.AxisListType.X)
                ecol = sb.tile([P, 1], f32, tag="ecol")
                nc.scalar.activation(ecol[:], scol[:],
                                     mybir.ActivationFunctionType.Exp)
                nc.scalar.activation(zst[:, l - l0, 0:D], hidden[:, l, :],
                                     mybir.ActivationFunctionType.Copy,
                                     scale=ecol[:])
                nc.vector.tensor_copy(zst[:, l - l0, D:D + 1], ecol[:])
                nc.vector.memset(zst[:, l - l0, D + 1:], 0.0)
            nc.sync.dma_start(zsv[:, l0:l1, :], zst[:, :l1 - l0, :])
        # zs writes must land before pool gathers read them
        tc.strict_bb_all_engine_barrier()

        # ---------- pooling ----------
        def pool(tb, idx_t, loc_t, psch):
            gsz_p, tok_p, mm_p, _, _ = psch
            off_p = np.zeros(NSB_G + 1, np.int64)
            off_p[1:] = np.cumsum(gsz_p)
            with tc.tile_pool(name=f"psp{tb}", bufs=4, space="PSUM") as psp, \
                    tc.tile_pool(name=f"gp{tb}", bufs=3) as gpool:
                tiles = {}
                hi_call = [-1]

                def need_call(s):
                    ci = s // SPCP
                    while hi_call[0] < ci:
                        cj = hi_call[0] + 1
                        n_ = min(PCALL, tok_p - cj * PCALL)
                        gt = gpool.tile([P, n_ // P, 2 * D], bf16, tag="gtp")
                        nc.gpsimd.dma_gather(
                            gt[:], zs[:],
                            idx_t[:, cj * PCALL // 16:(cj * PCALL + n_) // 16],
                            n_, n_, 2 * D, single_packet=False)
                        tiles[cj] = gt
                        tiles.pop(cj - 4, None)
                        hi_call[0] = cj
                    return tiles[ci]

                mcol = 0
                stg = None
                for sbk in range(NSB_G):
                    o, l = sbk // NSB, sbk % NSB
                    s0 = int(off_p[sbk]) // P
                    s1 = int(off_p[sbk] + gsz_p[sbk] - 1) // P
                    acc = psp.tile([P, D + 1], f32, tag="pacc")
                    for s in range(s0, s1 + 1):
                        gt = need_call(s)
                        oh = ohp.tile([P, P], bf16, tag="ohp")
                        eng = nc.gpsimd if mcol % 3 == 2 else nc.vector
                        eng.tensor_scalar(
                            out=oh[:], in0=iota_b[:],
                            scalar1=loc_t[:, mcol:mcol + 1],
                            scalar2=None, op0=mybir.AluOpType.is_equal)
                        nc.tensor.matmul(acc[:], lhsT=oh[:],
                                         rhs=gt[:, s % SPCP, 0:D + 1],
                                         start=s == s0, stop=s == s1)
                        mcol += 1
                    if l % 16 == 0:
                        stg = stp.tile([P, 16, D + 2], bf16, tag="pst")
                    nc.any.tensor_copy(stg[:, l % 16, 0:D + 1], acc[:])
                    if l % 16 == 15:
                        lb = l // 16 * 16
                        nc.sync.dma_start(
                            ppv[tb, o, :, lb:l + 1, 0:D + 2], stg[:, :, :])
                assert mcol == len(mm_p)
            if _DBG >= 7:
                rscatter(pool_part[tb * B:(tb + 1) * B, :],
                         rs_p[tb * BSEG:(tb + 1) * BSEG, :])

        if _DBG >= 6:
            pool(0, pool_idx_h, pool_loc_h, psch_h)
            pool(1, pool_idx_t, pool_loc_t, psch_t)

        # ---------- normalize + feats + MLP ----------
        with tc.tile_pool(name="psm", bufs=2, space="PSUM") as psm:
            for l in range(NSB if _DBG >= 8 else 1):
                fd = []
                for tb in range(2):
                    raw = sb.tile([P, D + 2], bf16, tag="raw")
                    nc.sync.dma_start(raw[:], rspv[tb, :, l, 0:D + 2])
                    den = sb.tile([P, 1], f32, tag="den")
                    nc.vector.tensor_scalar(out=den[:], in0=raw[:, D:D + 1],
                                            scalar1=1e-30, scalar2=None,
                                            op0=mybir.AluOpType.max)
                    deni = sb.tile([P, 1], f32, tag="deni")
                    nc.vector.reciprocal(deni[:], den[:])
                    pool_sl = sb.tile([P, D], f32, tag="psl")
                    nc.scalar.activation(pool_sl[:], raw[:, 0:D],
                                         mybir.ActivationFunctionType.Copy,
                                         scale=deni[:])
                    pt = psm.tile([P, D], f32, tag="pt")
                    nc.tensor.transpose(out=pt[:], in_=pool_sl[:],
                                        identity=ident[:])
                    fdt = sb.tile([P, D], bf16, tag=f"fd{tb}")
                    nc.any.tensor_copy(fdt[:], pt[:])
                    fd.append(fdt)
                ht = sb.tile([P, D], bf16, tag="fdht")
                nc.any.tensor_tensor(out=ht[:], in0=fd[0][:], in1=fd[1][:],
                                     op=mybir.AluOpType.mult)
                feats = [fd[0], fd[1], ht]

                o1 = sb.tile([P, 4, P], bf16, tag="o1")
                for m in range(4):
                    ps1 = psm.tile([P, P], f32, tag="ps1")
                    for kk in range(3):
                        nc.tensor.matmul(ps1[:],
                                         lhsT=w1t[:, kk, m * P:(m + 1) * P],
                                         rhs=feats[kk][:],
                                         start=kk == 0, stop=kk == 2)
                    nc.scalar.activation(o1[:, m, :], ps1[:],
                                         mybir.ActivationFunctionType.Relu,
                                         bias=b1t[:, m:m + 1])
                ps2 = psm.tile([R, P], f32, tag="ps2")
                for kk in range(4):
                    nc.tensor.matmul(ps2[:], lhsT=w2t[:, kk, :], rhs=o1[:, kk, :],
                                     start=kk == 0, stop=kk == 3)
                lg = sb.tile([R, P], f32, tag="lg")
                nc.vector.tensor_scalar(out=lg[:], in0=ps2[:], scalar1=b2t[:],
                                        scalar2=None, op0=mybir.AluOpType.add)
                lt = psm.tile([P, R], f32, tag="lt")
                nc.tensor.transpose(out=lt[:], in_=lg[:], identity=ident[:R, :R])
                lts = sb.tile([P, R], f32, tag="lts")
                nc.vector.tensor_copy(lts[:], lt[:])
                nc.sync.dma_start(
                    out.rearrange("(l p) r -> p l r", p=P)[:, l, :], lts[:])

    nc.compile()
    return nc


def kernel(embed, temp, attn_w, attn_b, W1, b1, W2, b2,
           edge_index, H_idx, H_seg, T_idx, T_seg, B):
    embed = np.asarray(embed, np.float32)
    temp = np.asarray(temp, np.float32)
    attn_w = np.asarray(attn_w, np.float32)
    W1 = np.asarray(W1, np.float32)
    b1 = np.asarray(b1, np.float32)
    W2 = np.asarray(W2, np.float32)
    b2 = np.asarray(b2, np.float32)
    edge_index = np.asarray(edge_index)
    H_idx, H_seg = np.asarray(H_idx), np.asarray(H_seg)
    T_idx, T_seg = np.asarray(T_idx), np.asarray(T_seg)

    src = edge_index[0].astype(np.int64)
    dst = edge_index[1].astype(np.int64)

    hsch = _prep_hop(src, dst)
    psch_h = _prep_pool(H_idx.astype(np.int64), H_seg.astype(np.int64))
    psch_t = _prep_pool(T_idx.astype(np.int64), T_seg.astype(np.int64))

    key = (hsch[0], psch_h[0], psch_t[0])
    if key not in _COMPILED:
        _COMPILED[key] = _build_program(hsch, psch_h, psch_t)
    nc = _COMPILED[key]

    deg_dst = np.bincount(dst, minlength=NP).astype(np.float64)
    deg_src = np.bincount(src, minlength=NP).astype(np.float64)
    a_full = (1.0 / np.sqrt(np.maximum(deg_src, 1.0))).astype(np.float32)
    b_full = (1.0 / np.sqrt(np.maximum(deg_dst, 1.0))).astype(np.float32)

    emb_full = np.zeros((NP, D), np.float32)
    emb_full[:N] = embed

    bf = ml_dtypes.bfloat16
    in_maps = []
    for c in range(NCORES):
        lo = c * SHARD
        a_c = a_full[lo:lo + SHARD].reshape(NBLK, P).T.copy()
        b_c = b_full[lo:lo + SHARD].reshape(NBLK, P).T.copy()
        bt_c = np.stack([b_c * temp[k + 1] for k in range(3)], axis=1)
        in_maps.append(dict(
            embed_in=np.ascontiguousarray(
                emb_full[lo:lo + SHARD].reshape(NBLK, P, D).transpose(1, 0, 2)),
            temp_in=np.tile(temp[None, :], (P, 1)),
            a_in=a_c,
            ab_in=a_c * b_c,
            bt_in=np.ascontiguousarray(bt_c),
            wrep_in=np.tile(attn_w[:, 0][None, :], (P, 1)),
            w1_in=W1.reshape(3, P, H_MLP).astype(bf),
            b1_in=np.ascontiguousarray(b1.reshape(4, P).T),
            w2_in=W2.reshape(4, P, R).astype(bf),
            b2_in=b2[:, None].copy(),
            hsrc=_wrap_idx16(hsch[3][c]),
            hloc=hsch[4][c],
            psrcH=_wrap_idx16(psch_h[3][c]),
            psrcT=_wrap_idx16(psch_t[3][c]),
            plocH=psch_h[4][c],
            plocT=psch_t[4][c],
        ))

    res = run_bass_kernel_spmd(nc, in_maps, list(range(NCORES)))
    return np.concatenate([res.results[c]["out"] for c in range(NCORES)], axis=0)


# revision 43
# speedup vs baseline: 1.1078x; 1.1078x over previous
import os
import sys

sys.path.insert(0, "/opt/trn_rl_repo")

_DBG = int(os.environ.get("KDBG", "9"))  # phase cutoff for fault bisection

from contextlib import ExitStack

import numpy as np
import ml_dtypes
import concourse.bacc as bacc
import concourse.bass as bass
import concourse.mybir as mybir
from concourse.bass_utils import run_bass_kernel_spmd
from concourse.tile import TileContext
from concourse.library_config import mlp as _mlp_lib
from concourse.masks import make_identity

P = 128
NCORES = 8
N, D, E, KHOP, B, L = 100000, 128, 1600000, 3, 32768, 262144
H_MLP, R = 512, 64
NBLK = 98              # local node blocks per core
SHARD = NBLK * P       # 12544 rows per core
NP = NCORES * SHARD    # 100352 padded rows
NBLK_G = NCORES * NBLK # 784 global dst blocks
NSB_G = B // P         # 256 global segment blocks
BSEG = B // NCORES     # 4096 segments per core
NSB = BSEG // P        # 32 local segment blocks
GCALL = 2048           # tokens per hop dma_gather call
SPC = GCALL // P       # slabs per hop gather call
PCALL = 2048           # tokens per pool dma_gather call (512B rows)
SPCP = PCALL // P      # slabs per pool gather call

f32 = mybir.dt.float32
bf16 = mybir.dt.bfloat16
i16 = mybir.dt.int16
i32 = mybir.dt.int32

_COMPILED = {}


def _wrap_idx16(idx):
    """dma_gather index layout: token i -> partition i%16, col i//16,
    replicated x8 to fill 128 partitions."""
    n = len(idx)
    assert n % 16 == 0
    return np.tile(idx.reshape(n // 16, 16).T.astype(np.int16), (8, 1))


def _grow(local):
    """Gather row of a local node index in the [p, l] table layout."""
    return (local & 127) * NBLK + (local >> 7)


def _schedule(per_core, nb, call):
    """per_core: list of (blk, loc, gidx) arrays, sorted by blk.
    Returns a shared straddled schedule plus per-core token/loc data."""
    cnts = np.zeros((NCORES, nb), np.int64)
    for c, (blk, _, _) in enumerate(per_core):
        cnts[c] = np.bincount(blk, minlength=nb)
    gsz = np.maximum(cnts.max(axis=0), 1)
    off = np.zeros(nb + 1, np.int64)
    off[1:] = np.cumsum(gsz)
    total = int(off[nb])
    tot_pad = (total + call - 1) // call * call
    mm = []  # (block, slab) in consumption order
    for t in range(nb):
        s0, s1 = off[t] // P, (off[t] + gsz[t] - 1) // P
        for s in range(s0, s1 + 1):
            mm.append((t, int(s)))
    M = len(mm)
    gstreams, loccols = [], []
    for c, (blk, loc, gidx) in enumerate(per_core):
        starts = np.concatenate([[0], np.cumsum(cnts[c])])
        pos = off[blk] + (np.arange(len(blk)) - starts[blk])
        g = np.zeros(tot_pad, np.int16)
        g[pos] = gidx
        lstream = np.full(tot_pad, -1.0, np.float32)
        lstream[pos] = loc
        cols = np.full((M, P), -1.0, np.float32)
        ar = np.arange(P)
        for m, (t, s) in enumerate(mm):
            idxs = s * P + ar
            seg = lstream[s * P:(s + 1) * P]
            mask = (idxs >= off[t]) & (idxs < off[t] + gsz[t])
            cols[m] = np.where(mask, seg, -1.0)
        gstreams.append(g)
        loccols.append(np.ascontiguousarray(cols.T))
    return tuple(int(v) for v in gsz), tot_pad, mm, gstreams, loccols


def _prep_hop(src, dst):
    per_core = []
    for c in range(NCORES):
        m = (src >= c * SHARD) & (src < (c + 1) * SHARD)
        sl = src[m] - c * SHARD
        dg = dst[m]
        t = dg >> 7
        order = np.argsort(t, kind="stable")
        t, sl, dg = t[order], sl[order], dg[order]
        per_core.append((t, (dg & 127).astype(np.float32),
                         _grow(sl).astype(np.int16)))
    return _schedule(per_core, NBLK_G, GCALL)


def _prep_pool(idx, seg):
    per_core = []
    for c in range(NCORES):
        m = (idx >= c * SHARD) & (idx < (c + 1) * SHARD)
        r = idx[m] - c * SHARD
        sg = seg[m]
        sb = sg >> 7
        order = np.argsort(sb, kind="stable")
        sb, r, sg = sb[order], r[order], sg[order]
        per_core.append((sb, (sg & 127).astype(np.float32),
                         _grow(r).astype(np.int16)))
    return _schedule(per_core, NSB_G, PCALL)


def _build_program(hsch, psch_h, psch_t):
    nc = bacc.Bacc("TRN2", target_bir_lowering=False, num_devices=NCORES)

    gsz_h, tok_h, mm_h, _, _ = hsch
    gsz_ph, tok_ph, mm_ph, _, _ = psch_h
    gsz_pt, tok_pt, mm_pt, _, _ = psch_t
    off_h = np.zeros(NBLK_G + 1, np.int64)
    off_h[1:] = np.cumsum(gsz_h)

    embed_in = nc.dram_tensor("embed_in", [P, NBLK, D], f32, kind="ExternalInput")
    temp_in = nc.dram_tensor("temp_in", [P, 4], f32, kind="ExternalInput")
    a_in = nc.dram_tensor("a_in", [P, NBLK], f32, kind="ExternalInput")
    ab_in = nc.dram_tensor("ab_in", [P, NBLK], f32, kind="ExternalInput")
    bt_in = nc.dram_tensor("bt_in", [P, 3, NBLK], f32, kind="ExternalInput")
    wrep_in = nc.dram_tensor("wrep_in", [P, D], f32, kind="ExternalInput")
    w1_in = nc.dram_tensor("w1_in", [3, P, H_MLP], bf16, kind="ExternalInput")
    b1_in = nc.dram_tensor("b1_in", [P, 4], f32, kind="ExternalInput")
    w2_in = nc.dram_tensor("w2_in", [4, P, R], bf16, kind="ExternalInput")
    b2_in = nc.dram_tensor("b2_in", [R, 1], f32, kind="ExternalInput")
    hsrc = nc.dram_tensor("hsrc", [P, tok_h // 16], i16, kind="ExternalInput")
    hloc = nc.dram_tensor("hloc", [P, len(mm_h)], f32, kind="ExternalInput")
    psrcH = nc.dram_tensor("psrcH", [P, tok_ph // 16], i16, kind="ExternalInput")
    psrcT = nc.dram_tensor("psrcT", [P, tok_pt // 16], i16, kind="ExternalInput")
    plocH = nc.dram_tensor("plocH", [P, len(mm_ph)], f32, kind="ExternalInput")
    plocT = nc.dram_tensor("plocT", [P, len(mm_pt)], f32, kind="ExternalInput")

    out = nc.dram_tensor("out", [BSEG, R], f32, kind="ExternalOutput")

    xs_a = nc.dram_tensor("xs_a", [SHARD, D], bf16)
    xs_b = nc.dram_tensor("xs_b", [SHARD, D], bf16)
    part_a = nc.dram_tensor("part_a", [NP, D], bf16)
    part_b = nc.dram_tensor("part_b", [NP, D], bf16)
    rsx = nc.dram_tensor("rsx", [SHARD, D], bf16)
    zs = nc.dram_tensor("zs", [SHARD, 2 * D], bf16)
    pool_part = nc.dram_tensor("pool_part", [2 * B, 2 * D], bf16)
    rs_p = nc.dram_tensor("rs_p", [2 * BSEG, 2 * D], bf16)

    rg = [list(range(NCORES))]

    def xview(t):  # [p, l, d] view of a [SHARD, D] table
        return t.rearrange("(p l) d -> p l d", p=P)

    part_av = part_a.rearrange("(o p l) d -> o p l d", o=NCORES, p=P)
    part_bv = part_b.rearrange("(o p l) d -> o p l d", o=NCORES, p=P)
    ppv = pool_part.rearrange("(t o s l) d -> t o s l d", t=2, o=NCORES, s=P)
    rspv = rs_p.rearrange("(t s l) d -> t s l d", t=2, s=P)

    with TileContext(nc) as tc, ExitStack() as ctx:
        sb = ctx.enter_context(tc.tile_pool(name="sb", bufs=3))
        const = ctx.enter_context(tc.tile_pool(name="const", bufs=1))
        ohp = ctx.enter_context(tc.tile_pool(name="ohp", bufs=6))
        ccs = ctx.enter_context(nc.semaphore("ccs"))
        ccs_val = [0]

        def rscatter(ins_ap, outs_ap):
            tc.strict_bb_all_engine_barrier()
            with tc.tile_critical():
                ccs_val[0] += 1
                nc.gpsimd.collective_compute(
                    "ReduceScatter", mybir.AluOpType.add,
                    ins=[ins_ap], outs=[outs_ap], replica_groups=rg,
                ).then_inc(ccs, 1)
                nc.gpsimd.wait_ge(ccs, ccs_val[0])
            tc.strict_bb_all_engine_barrier()

        nc.gpsimd.load_library(_mlp_lib)

        # ---------- constants ----------
        iota_i = const.tile([P, P], i32)
        nc.gpsimd.iota(iota_i[:], pattern=[[1, P]], base=0, channel_multiplier=0)
        iota_b = const.tile([P, P], bf16)
        nc.vector.tensor_copy(iota_b[:], iota_i[:])
        ident = const.tile([P, P], f32)
        make_identity(nc, ident[:])

        temp_sb = const.tile([P, 4], f32)
        nc.sync.dma_start(temp_sb[:], temp_in[:])
        a_sc = const.tile([P, NBLK], f32)
        nc.sync.dma_start(a_sc[:], a_in[:])
        ab_sc = const.tile([P, NBLK], f32)
        nc.sync.dma_start(ab_sc[:], ab_in[:])
        bt_sc = const.tile([P, 3, NBLK], f32)
        nc.sync.dma_start(bt_sc[:], bt_in[:])
        wrep = const.tile([P, D], f32)
        nc.sync.dma_start(wrep[:], wrep_in[:])
        w1t = const.tile([P, 3, H_MLP], bf16)
        nc.sync.dma_start(w1t[:], w1_in.rearrange("k p h -> p k h")[:])
        b1t = const.tile([P, 4], f32)
        nc.sync.dma_start(b1t[:], b1_in[:])
        w2t = const.tile([P, 4, R], bf16)
        nc.sync.dma_start(w2t[:], w2_in.rearrange("k p r -> p k r")[:])
        b2t = const.tile([R, 1], f32)
        nc.sync.dma_start(b2t[:], b2_in[:])

        hop_idx = const.tile([P, tok_h // 16], i16)
        nc.sync.dma_start(hop_idx[:], hsrc[:])
        hop_loc = const.tile([P, len(mm_h)], f32)
        nc.sync.dma_start(hop_loc[:], hloc[:])
        pool_idx_h = const.tile([P, tok_ph // 16], i16)
        nc.sync.dma_start(pool_idx_h[:], psrcH[:])
        pool_idx_t = const.tile([P, tok_pt // 16], i16)
        nc.sync.dma_start(pool_idx_t[:], psrcT[:])
        pool_loc_h = const.tile([P, len(mm_ph)], f32)
        nc.sync.dma_start(pool_loc_h[:], plocH[:])
        pool_loc_t = const.tile([P, len(mm_pt)], f32)
        nc.sync.dma_start(pool_loc_t[:], plocT[:])

        # ---------- init: hidden = temp0*embed, xs_a = a_sc*embed ----------
        stp = ctx.enter_context(tc.tile_pool(name="stp", bufs=3))
        hidden = const.tile([P, NBLK, D], f32)
        nc.sync.dma_start(hidden[:], embed_in[:])
        for l0 in range(0, NBLK, 16):
            l1 = min(l0 + 16, NBLK)
            stg = stp.tile([P, 16, D], bf16, tag="xst")
            for l in range(l0, l1):
                nc.any.tensor_scalar(out=stg[:, l - l0, :], in0=hidden[:, l, :],
                                     scalar1=a_sc[:, l:l + 1], scalar2=None,
                                     op0=mybir.AluOpType.mult)
            nc.sync.dma_start(xview(xs_a)[:, l0:l1, :], stg[:, :l1 - l0, :])
        nc.vector.tensor_scalar(out=hidden[:], in0=hidden[:],
                                scalar1=temp_sb[:, 0:1], scalar2=None,
                                op0=mybir.AluOpType.mult)

        # ---------- hops ----------
        # precompute per-block slab spans
        spans = []
        for t in range(NBLK_G):
            s0 = int(off_h[t]) // P
            s1 = int(off_h[t] + gsz_h[t] - 1) // P
            spans.append((s0, s1))

        with tc.tile_pool(name="psh", bufs=6, space="PSUM") as psh, \
                tc.tile_pool(name="ghop", bufs=3) as gpool:
            for k in range(KHOP if _DBG >= 4 else (1 if _DBG >= 2 else 0)):
                xs_src = xs_a if k % 2 == 0 else xs_b
                xs_dst = xs_b if k % 2 == 0 else xs_a
                part = part_a if k % 2 == 0 else part_b
                partv = part_av if k % 2 == 0 else part_bv

                # xs_src DRAM writes (init / previous post-phase) must land
                # before this hop's gathers read them (DRAM is untracked).
                tc.strict_bb_all_engine_barrier()

                tiles = {}
                hi_call = [-1]

                def need_call(s, xs_src=xs_src, tiles=tiles, hi_call=hi_call,
                              toktot=tok_h, idx_t=hop_idx, tabsrc=None, w=D):
                    ci = s // SPC
                    while hi_call[0] < ci:
                        cj = hi_call[0] + 1
                        n_ = min(GCALL, toktot - cj * GCALL)
                        gt = gpool.tile([P, n_ // P, w], bf16, tag="gt")
                        nc.gpsimd.dma_gather(
                            gt[:], xs_src[:],
                            idx_t[:, cj * GCALL // 16:(cj * GCALL + n_) // 16],
                            n_, n_, w, single_packet=False)
                        tiles[cj] = gt
                        tiles.pop(cj - 4, None)
                        hi_call[0] = cj
                    return tiles[ci]

                mcol = 0
                stg = None
                for t in range(NBLK_G):
                    o, l = t // NBLK, t % NBLK
                    s0, s1 = spans[t]
                    acc = psh.tile([P, D], f32, tag="acc")
                    for s in range(s0, s1 + 1):
                        gt = need_call(s)
                        oh = ohp.tile([P, P], bf16, tag="oh")
                        nc.any.tensor_scalar(
                            out=oh[:], in0=iota_b[:],
                            scalar1=hop_loc[:, mcol:mcol + 1],
                            scalar2=None, op0=mybir.AluOpType.is_equal)
                        nc.tensor.matmul(acc[:], lhsT=oh[:],
                                         rhs=gt[:, s % SPC, :],
                                         start=s == s0, stop=s == s1)
                        mcol += 1
                    if l % 16 == 0:
                        stg = stp.tile([P, 16, D], bf16, tag="fst")
                    nc.any.tensor_copy(stg[:, l % 16, :], acc[:])
                    if l % 16 == 15 or l == NBLK - 1:
                        lb = l // 16 * 16
                        nc.sync.dma_start(partv[o, :, lb:l + 1, :],
                                          stg[:, :l - lb + 1, :])
                assert mcol == len(mm_h)

                # reduce-scatter partial -> own shard; update hidden/xs.
                # xs_dst writes are emitted first (they gate the next hop's
                # gathers via the top-of-hop barrier); the hidden updates are
                # appended after and overlap the next hop's compute.
                if _DBG < 3:
                    continue
                rscatter(part[:], rsx[:])
                for l0 in range(0, NBLK, 16):
                    l1 = min(l0 + 16, NBLK)
                    rsb = sb.tile([P, 16, D], bf16, tag="rsb")
                    nc.sync.dma_start(rsb[:, :l1 - l0, :],
                                      xview(rsx)[:, l0:l1, :])
                    if k < KHOP - 1:
                        stg2 = stp.tile([P, 16, D], bf16, tag="xst")
                    for l in range(l0, l1):
                        tmp = sb.tile([P, D], f32, tag="tmp")
                        nc.any.tensor_scalar(out=tmp[:], in0=rsb[:, l - l0, :],
                                             scalar1=bt_sc[:, k, l:l + 1],
                                             scalar2=None,
                                             op0=mybir.AluOpType.mult)
                        nc.any.tensor_tensor(out=hidden[:, l, :],
                                             in0=hidden[:, l, :], in1=tmp[:],
                                             op=mybir.AluOpType.add)
                        if k < KHOP - 1:
                            nc.any.tensor_scalar(out=stg2[:, l - l0, :],
                                                 in0=rsb[:, l - l0, :],
                                                 scalar1=ab_sc[:, l:l + 1],
                                                 scalar2=None,
                                                 op0=mybir.AluOpType.mult)
                    if k < KHOP - 1:
                        nc.sync.dma_start(xview(xs_dst)[:, l0:l1, :],
                                          stg2[:, :l1 - l0, :])

        # ---------- z_ext = [z*e | e | junk] to zs ----------
        zsv = zs.rearrange("(p l) d -> p l d", p=P)
        for l0 in range(0 if _DBG >= 5 else NBLK, NBLK, 8):
            l1 = min(l0 + 8, NBLK)
            zst = stp.tile([P, 8, 2 * D], bf16, tag="zst")
            for l in range(l0, l1):
                prod = sb.tile([P, D], f32, tag="prod")
                nc.any.tensor_tensor(out=prod[:], in0=hidden[:, l, :],
                                     in1=wrep[:], op=mybir.AluOpType.mult)
                scol = sb.tile([P, 1], f32, tag="scol")
                nc.vector.reduce_sum(scol[:], prod[:], axis=mybir.AxisListType.X)
                ecol = sb.tile([P, 1], f32, tag="ecol")
                nc.scalar.activation(ecol[:], scol[:],
                                     mybir.ActivationFunctionType.Exp)
                nc.any.tensor_scalar(out=zst[:, l - l0, 0:D],
                                     in0=hidden[:, l, :], scalar1=ecol[:],
                                     scalar2=None, op0=mybir.AluOpType.mult)
                nc.vector.tensor_copy(zst[:, l - l0, D:D + 1], ecol[:])
                nc.vector.memset(zst[:, l - l0, D + 1:], 0.0)
            nc.sync.dma_start(zsv[:, l0:l1, :], zst[:, :l1 - l0, :])
        # zs writes must land before pool gathers read them
        tc.strict_bb_all_engine_barrier()

        # ---------- pooling ----------
        def pool(tb, idx_t, loc_t, psch):
            gsz_p, tok_p, mm_p, _, _ = psch
            off_p = np.zeros(NSB_G + 1, np.int64)
            off_p[1:] = np.cumsum(gsz_p)
            with tc.tile_pool(name=f"psp{tb}", bufs=4, space="PSUM") as psp, \
                    tc.tile_pool(name=f"gp{tb}", bufs=3) as gpool:
                tiles = {}
                hi_call = [-1]

                def need_call(s):
                    ci = s // SPCP
                    while hi_call[0] < ci:
                        cj = hi_call[0] + 1
                        n_ = min(PCALL, tok_p - cj * PCALL)
                        gt = gpool.tile([P, n_ // P, 2 * D], bf16, tag="gtp")
                        nc.gpsimd.dma_gather(
                            gt[:], zs[:],
                            idx_t[:, cj * PCALL // 16:(cj * PCALL + n_) // 16],
                            n_, n_, 2 * D, single_packet=False)
                        tiles[cj] = gt
                        tiles.pop(cj - 4, None)
                        hi_call[0] = cj
                    return tiles[ci]

                mcol = 0
                stg = None
                for sbk in range(NSB_G):
                    o, l = sbk // NSB, sbk % NSB
                    s0 = int(off_p[sbk]) // P
                    s1 = int(off_p[sbk] + gsz_p[sbk] - 1) // P
                    acc = psp.tile([P, D + 1], f32, tag="pacc")
                    for s in range(s0, s1 + 1):
                        gt = need_call(s)
                        oh = ohp.tile([P, P], bf16, tag="ohp")
                        nc.any.tensor_scalar(
                            out=oh[:], in0=iota_b[:],
                            scalar1=loc_t[:, mcol:mcol + 1],
                            scalar2=None, op0=mybir.AluOpType.is_equal)
                        nc.tensor.matmul(acc[:], lhsT=oh[:],
                                         rhs=gt[:, s % SPCP, 0:D + 1],
                                         start=s == s0, stop=s == s1)
                        mcol += 1
                    if l % 16 == 0:
                        stg = stp.tile([P, 16, D + 2], bf16, tag="pst")
                    nc.any.tensor_copy(stg[:, l % 16, 0:D + 1], acc[:])
                    if l % 16 == 15:
                        lb = l // 16 * 16
                        nc.sync.dma_start(
                            ppv[tb, o, :, lb:l + 1, 0:D + 2], stg[:, :, :])
                assert mcol == len(mm_p)
            if _DBG >= 7:
                rscatter(pool_part[tb * B:(tb + 1) * B, :],
                         rs_p[tb * BSEG:(tb + 1) * BSEG, :])

        if _DBG >= 6:
            pool(0, pool_idx_h, pool_loc_h, psch_h)
            pool(1, pool_idx_t, pool_loc_t, psch_t)

        # ---------- normalize + feats + MLP ----------
        with tc.tile_pool(name="psm", bufs=2, space="PSUM") as psm:
            for l in range(NSB if _DBG >= 8 else 1):
                fd = []
                for tb in range(2):
                    raw = sb.tile([P, D + 2], bf16, tag="raw")
                    nc.sync.dma_start(raw[:], rspv[tb, :, l, 0:D + 2])
                    den = sb.tile([P, 1], f32, tag="den")
                    nc.vector.tensor_scalar(out=den[:], in0=raw[:, D:D + 1],
                                            scalar1=1e-30, scalar2=None,
                                            op0=mybir.AluOpType.max)
                    deni = sb.tile([P, 1], f32, tag="deni")
                    nc.vector.reciprocal(deni[:], den[:])
                    pool_sl = sb.tile([P, D], f32, tag="psl")
                    nc.any.tensor_scalar(out=pool_sl[:], in0=raw[:, 0:D],
                                         scalar1=deni[:], scalar2=None,
                                         op0=mybir.AluOpType.mult)
                    pt = psm.tile([P, D], f32, tag="pt")
                    nc.tensor.transpose(out=pt[:], in_=pool_sl[:],
                                        identity=ident[:])
                    fdt = sb.tile([P, D], bf16, tag=f"fd{tb}")
                    nc.any.tensor_copy(fdt[:], pt[:])
                    fd.append(fdt)
                ht = sb.tile([P, D], bf16, tag="fdht")
                nc.any.tensor_tensor(out=ht[:], in0=fd[0][:], in1=fd[1][:],
                                     op=mybir.AluOpType.mult)
                feats = [fd[0], fd[1], ht]

                o1 = sb.tile([P, 4, P], bf16, tag="o1")
                for m in range(4):
                    ps1 = psm.tile([P, P], f32, tag="ps1")
                    for kk in range(3):
                        nc.tensor.matmul(ps1[:],
                                         lhsT=w1t[:, kk, m * P:(m + 1) * P],
                                         rhs=feats[kk][:],
                                         start=kk == 0, stop=kk == 2)
                    nc.scalar.activation(o1[:, m, :], ps1[:],
                                         mybir.ActivationFunctionType.Relu,
                                         bias=b1t[:, m:m + 1])
                ps2 = psm.tile([R, P], f32, tag="ps2")
                for kk in range(4):
                    nc.tensor.matmul(ps2[:], lhsT=w2t[:, kk, :], rhs=o1[:, kk, :],
                                     start=kk == 0, stop=kk == 3)
                lg = sb.tile([R, P], f32, tag="lg")
                nc.vector.tensor_scalar(out=lg[:], in0=ps2[:], scalar1=b2t[:],
                                        scalar2=None, op0=mybir.AluOpType.add)
                lt = psm.tile([P, R], f32, tag="lt")
                nc.tensor.transpose(out=lt[:], in_=lg[:], identity=ident[:R, :R])
                lts = sb.tile([P, R], f32, tag="lts")
                nc.vector.tensor_copy(lts[:], lt[:])
                nc.sync.dma_start(
                    out.rearrange("(l p) r -> p l r", p=P)[:, l, :], lts[:])

    nc.compile()
    return nc


def kernel(embed, temp, attn_w, attn_b, W1, b1, W2, b2,
           edge_index, H_idx, H_seg, T_idx, T_seg, B):
    embed = np.asarray(embed, np.float32)
    temp = np.asarray(temp, np.float32)
    attn_w = np.asarray(attn_w, np.float32)
    W1 = np.asarray(W1, np.float32)
    b1 = np.asarray(b1, np.float32)
    W2 = np.asarray(W2, np.float32)
    b2 = np.asarray(b2, np.float32)
    edge_index = np.asarray(edge_index)
    H_idx, H_seg = np.asarray(H_idx), np.asarray(H_seg)
    T_idx, T_seg = np.asarray(T_idx), np.asarray(T_seg)

    src = edge_index[0].astype(np.int64)
    dst = edge_index[1].astype(np.int64)

    hsch = _prep_hop(src, dst)
    psch_h = _prep_pool(H_idx.astype(np.int64), H_seg.astype(np.int64))
    psch_t = _prep_pool(T_idx.astype(np.int64), T_seg.astype(np.int64))

    key = (hsch[0], psch_h[0], psch_t[0])
    if key not in _COMPILED:
        _COMPILED[key] = _build_program(hsch, psch_h, psch_t)
    nc = _COMPILED[key]

    deg_dst = np.bincount(dst, minlength=NP).astype(np.float64)
    deg_src = np.bincount(src, minlength=NP).astype(np.float64)
    a_full = (1.0 / np.sqrt(np.maximum(deg_src, 1.0))).astype(np.float32)
    b_full = (1.0 / np.sqrt(np.maximum(deg_dst, 1.0))).astype(np.float32)

    emb_full = np.zeros((NP, D), np.float32)
    emb_full[:N] = embed

    bf = ml_dtypes.bfloat16
    in_maps = []
    for c in range(NCORES):
        lo = c * SHARD
        a_c = a_full[lo:lo + SHARD].reshape(NBLK, P).T.copy()
        b_c = b_full[lo:lo + SHARD].reshape(NBLK, P).T.copy()
        bt_c = np.stack([b_c * temp[k + 1] for k in range(3)], axis=1)
        in_maps.append(dict(
            embed_in=np.ascontiguousarray(
                emb_full[lo:lo + SHARD].reshape(NBLK, P, D).transpose(1, 0, 2)),
            temp_in=np.tile(temp[None, :], (P, 1)),
            a_in=a_c,
            ab_in=a_c * b_c,
            bt_in=np.ascontiguousarray(bt_c),
            wrep_in=np.tile(attn_w[:, 0][None, :], (P, 1)),
            w1_in=W1.reshape(3, P, H_MLP).astype(bf),
            b1_in=np.ascontiguousarray(b1.reshape(4, P).T),
            w2_in=W2.reshape(4, P, R).astype(bf),
            b2_in=b2[:, None].copy(),
            hsrc=_wrap_idx16(hsch[3][c]),
            hloc=hsch[4][c],
            psrcH=_wrap_idx16(psch_h[3][c]),
            psrcT=_wrap_idx16(psch_t[3][c]),
            plocH=psch_h[4][c],
            plocT=psch_t[4][c],
        ))

    res = run_bass_kernel_spmd(nc, in_maps, list(range(NCORES)))
    return np.concatenate([res.results[c]["out"] for c in range(NCORES)], axis=0)


# revision 46
# speedup vs baseline: 1.1446x; 1.0332x over previous
import os
import sys

sys.path.insert(0, "/opt/trn_rl_repo")

_DBG = int(os.environ.get("KDBG", "9"))  # phase cutoff for fault bisection

from contextlib import ExitStack

import numpy as np
import ml_dtypes
import concourse.bacc as bacc
import concourse.bass as bass
import concourse.mybir as mybir
from concourse.bass_utils import run_bass_kernel_spmd
from concourse.tile import TileContext
from concourse.library_config import mlp as _mlp_lib
from concourse.masks import make_identity

P = 128
NCORES = 8
N, D, E, KHOP, B, L = 100000, 128, 1600000, 3, 32768, 262144
H_MLP, R = 512, 64
NBLK = 98              # local node blocks per core
SHARD = NBLK * P       # 12544 rows per core
NP = NCORES * SHARD    # 100352 padded rows
NBLK_G = NCORES * NBLK # 784 global dst blocks
NSB_G = B // P         # 256 global segment blocks
BSEG = B // NCORES     # 4096 segments per core
NSB = BSEG // P        # 32 local segment blocks
GCALL = 2048           # tokens per hop dma_gather call
SPC = GCALL // P       # slabs per hop gather call
PCALL = 2048           # tokens per pool dma_gather call (512B rows)
SPCP = PCALL // P      # slabs per pool gather call

f32 = mybir.dt.float32
bf16 = mybir.dt.bfloat16
i16 = mybir.dt.int16
i32 = mybir.dt.int32

_COMPILED = {}


def _wrap_idx16(idx):
    """dma_gather index layout: token i -> partition i%16, col i//16,
    replicated x8 to fill 128 partitions."""
    n = len(idx)
    assert n % 16 == 0
    return np.tile(idx.reshape(n // 16, 16).T.astype(np.int16), (8, 1))


def _grow(local):
    """Gather row of a local node index in the [p, l] table layout."""
    return (local & 127) * NBLK + (local >> 7)


def _schedule(per_core, nb, call):
    """per_core: list of (blk, loc, gidx) arrays, sorted by blk.
    Returns a shared straddled schedule plus per-core token/loc data."""
    cnts = np.zeros((NCORES, nb), np.int64)
    for c, (blk, _, _) in enumerate(per_core):
        cnts[c] = np.bincount(blk, minlength=nb)
    gsz = np.maximum(cnts.max(axis=0), 1)
    off = np.zeros(nb + 1, np.int64)
    off[1:] = np.cumsum(gsz)
    total = int(off[nb])
    tot_pad = (total + call - 1) // call * call
    mm = []  # (block, slab) in consumption order
    for t in range(nb):
        s0, s1 = off[t] // P, (off[t] + gsz[t] - 1) // P
        for s in range(s0, s1 + 1):
            mm.append((t, int(s)))
    M = len(mm)
    gstreams, loccols = [], []
    for c, (blk, loc, gidx) in enumerate(per_core):
        starts = np.concatenate([[0], np.cumsum(cnts[c])])
        pos = off[blk] + (np.arange(len(blk)) - starts[blk])
        g = np.zeros(tot_pad, np.int16)
        g[pos] = gidx
        lstream = np.full(tot_pad, -1.0, np.float32)
        lstream[pos] = loc
        cols = np.full((M, P), -1.0, np.float32)
        ar = np.arange(P)
        for m, (t, s) in enumerate(mm):
            idxs = s * P + ar
            seg = lstream[s * P:(s + 1) * P]
            mask = (idxs >= off[t]) & (idxs < off[t] + gsz[t])
            cols[m] = np.where(mask, seg, -1.0)
        gstreams.append(g)
        loccols.append(np.ascontiguousarray(cols.T))
    return tuple(int(v) for v in gsz), tot_pad, mm, gstreams, loccols


def _prep_hop(src, dst):
    per_core = []
    for c in range(NCORES):
        m = (src >= c * SHARD) & (src < (c + 1) * SHARD)
        sl = src[m] - c * SHARD
        dg = dst[m]
        t = dg >> 7
        order = np.argsort(t, kind="stable")
        t, sl, dg = t[order], sl[order], dg[order]
        per_core.append((t, (dg & 127).astype(np.float32),
                         _grow(sl).astype(np.int16)))
    return _schedule(per_core, NBLK_G, GCALL)


def _prep_pool(idx, seg):
    per_core = []
    for c in range(NCORES):
        m = (idx >= c * SHARD) & (idx < (c + 1) * SHARD)
        r = idx[m] - c * SHARD
        sg = seg[m]
        sb = sg >> 7
        order = np.argsort(sb, kind="stable")
        sb, r, sg = sb[order], r[order], sg[order]
        per_core.append((sb, (sg & 127).astype(np.float32),
                         _grow(r).astype(np.int16)))
    return _schedule(per_core, NSB_G, PCALL)


def _build_program(hsch, psch_h, psch_t):
    nc = bacc.Bacc("TRN2", target_bir_lowering=False, num_devices=NCORES)

    gsz_h, tok_h, mm_h, _, _ = hsch
    gsz_ph, tok_ph, mm_ph, _, _ = psch_h
    gsz_pt, tok_pt, mm_pt, _, _ = psch_t
    off_h = np.zeros(NBLK_G + 1, np.int64)
    off_h[1:] = np.cumsum(gsz_h)

    embed_in = nc.dram_tensor("embed_in", [P, NBLK, D], f32, kind="ExternalInput")
    temp_in = nc.dram_tensor("temp_in", [P, 4], f32, kind="ExternalInput")
    a_in = nc.dram_tensor("a_in", [P, NBLK], f32, kind="ExternalInput")
    ab_in = nc.dram_tensor("ab_in", [P, NBLK], f32, kind="ExternalInput")
    bt_in = nc.dram_tensor("bt_in", [P, 3, NBLK], f32, kind="ExternalInput")
    wrep_in = nc.dram_tensor("wrep_in", [P, D], f32, kind="ExternalInput")
    w1_in = nc.dram_tensor("w1_in", [3, P, H_MLP], bf16, kind="ExternalInput")
    b1_in = nc.dram_tensor("b1_in", [P, 4], f32, kind="ExternalInput")
    w2_in = nc.dram_tensor("w2_in", [4, P, R], bf16, kind="ExternalInput")
    b2_in = nc.dram_tensor("b2_in", [R, 1], f32, kind="ExternalInput")
    hsrc = nc.dram_tensor("hsrc", [P, tok_h // 16], i16, kind="ExternalInput")
    hloc = nc.dram_tensor("hloc", [P, len(mm_h)], f32, kind="ExternalInput")
    psrcH = nc.dram_tensor("psrcH", [P, tok_ph // 16], i16, kind="ExternalInput")
    psrcT = nc.dram_tensor("psrcT", [P, tok_pt // 16], i16, kind="ExternalInput")
    plocH = nc.dram_tensor("plocH", [P, len(mm_ph)], f32, kind="ExternalInput")
    plocT = nc.dram_tensor("plocT", [P, len(mm_pt)], f32, kind="ExternalInput")

    out = nc.dram_tensor("out", [BSEG, R], f32, kind="ExternalOutput")

    xs_a = nc.dram_tensor("xs_a", [SHARD, D], bf16)
    xs_b = nc.dram_tensor("xs_b", [SHARD, D], bf16)
    part_a = nc.dram_tensor("part_a", [NP, D], bf16)
    part_b = nc.dram_tensor("part_b", [NP, D], bf16)
    rsx = nc.dram_tensor("rsx", [SHARD, D], bf16)
    zs = nc.dram_tensor("zs", [SHARD, 2 * D], bf16)
    pool_part = nc.dram_tensor("pool_part", [2 * B, 2 * D], bf16)
    rs_p = nc.dram_tensor("rs_p", [2 * BSEG, 2 * D], bf16)

    rg = [list(range(NCORES))]

    def xview(t):  # [p, l, d] view of a [SHARD, D] table
        return t.rearrange("(p l) d -> p l d", p=P)

    part_av = part_a.rearrange("(o p l) d -> o p l d", o=NCORES, p=P)
    part_bv = part_b.rearrange("(o p l) d -> o p l d", o=NCORES, p=P)
    ppv = pool_part.rearrange("(t o s l) d -> t o s l d", t=2, o=NCORES, s=P)
    rspv = rs_p.rearrange("(t s l) d -> t s l d", t=2, s=P)

    with TileContext(nc) as tc, ExitStack() as ctx:
        sb = ctx.enter_context(tc.tile_pool(name="sb", bufs=3))
        const = ctx.enter_context(tc.tile_pool(name="const", bufs=1))
        ohp = ctx.enter_context(tc.tile_pool(name="ohp", bufs=6))
        ccs = ctx.enter_context(nc.semaphore("ccs"))
        pfs = ctx.enter_context(nc.semaphore("pfs"))
        ccs_val = [0]
        pfs_val = [0]

        def rscatter(ins_ap, outs_ap):
            tc.strict_bb_all_engine_barrier()
            with tc.tile_critical():
                ccs_val[0] += 1
                nc.gpsimd.collective_compute(
                    "ReduceScatter", mybir.AluOpType.add,
                    ins=[ins_ap], outs=[outs_ap], replica_groups=rg,
                ).then_inc(ccs, 1)
                nc.gpsimd.wait_ge(ccs, ccs_val[0])
            tc.strict_bb_all_engine_barrier()

        nc.gpsimd.load_library(_mlp_lib)

        # ---------- constants ----------
        iota_i = const.tile([P, P], i32)
        nc.gpsimd.iota(iota_i[:], pattern=[[1, P]], base=0, channel_multiplier=0)
        iota_b = const.tile([P, P], bf16)
        nc.vector.tensor_copy(iota_b[:], iota_i[:])
        ident = const.tile([P, P], f32)
        make_identity(nc, ident[:])

        temp_sb = const.tile([P, 4], f32)
        nc.sync.dma_start(temp_sb[:], temp_in[:])
        a_sc = const.tile([P, NBLK], f32)
        nc.sync.dma_start(a_sc[:], a_in[:])
        ab_sc = const.tile([P, NBLK], f32)
        nc.sync.dma_start(ab_sc[:], ab_in[:])
        bt_sc = const.tile([P, 3, NBLK], f32)
        nc.sync.dma_start(bt_sc[:], bt_in[:])
        wrep = const.tile([P, D], f32)
        nc.sync.dma_start(wrep[:], wrep_in[:])
        w1t = const.tile([P, 3, H_MLP], bf16)
        nc.sync.dma_start(w1t[:], w1_in.rearrange("k p h -> p k h")[:])
        b1t = const.tile([P, 4], f32)
        nc.sync.dma_start(b1t[:], b1_in[:])
        w2t = const.tile([P, 4, R], bf16)
        nc.sync.dma_start(w2t[:], w2_in.rearrange("k p r -> p k r")[:])
        b2t = const.tile([R, 1], f32)
        nc.sync.dma_start(b2t[:], b2_in[:])

        hop_idx = const.tile([P, tok_h // 16], i16)
        nc.sync.dma_start(hop_idx[:], hsrc[:])
        hop_loc = const.tile([P, len(mm_h)], f32)
        nc.sync.dma_start(hop_loc[:], hloc[:])
        pool_idx_h = const.tile([P, tok_ph // 16], i16)
        nc.sync.dma_start(pool_idx_h[:], psrcH[:])
        pool_idx_t = const.tile([P, tok_pt // 16], i16)
        nc.sync.dma_start(pool_idx_t[:], psrcT[:])
        pool_loc_h = const.tile([P, len(mm_ph)], f32)
        nc.sync.dma_start(pool_loc_h[:], plocH[:])
        pool_loc_t = const.tile([P, len(mm_pt)], f32)
        nc.sync.dma_start(pool_loc_t[:], plocT[:])

        # ---------- init: hidden = temp0*embed, xs_a = a_sc*embed ----------
        stp = ctx.enter_context(tc.tile_pool(name="stp", bufs=3))
        hidden = const.tile([P, NBLK, D], f32)
        nc.sync.dma_start(hidden[:], embed_in[:])
        for l0 in range(0, NBLK, 16):
            l1 = min(l0 + 16, NBLK)
            stg = stp.tile([P, 16, D], bf16, tag="xst")
            for l in range(l0, l1):
                nc.any.tensor_scalar(out=stg[:, l - l0, :], in0=hidden[:, l, :],
                                     scalar1=a_sc[:, l:l + 1], scalar2=None,
                                     op0=mybir.AluOpType.mult)
            nc.sync.dma_start(xview(xs_a)[:, l0:l1, :], stg[:, :l1 - l0, :])
        nc.vector.tensor_scalar(out=hidden[:], in0=hidden[:],
                                scalar1=temp_sb[:, 0:1], scalar2=None,
                                op0=mybir.AluOpType.mult)

        # ---------- hops ----------
        # precompute per-block slab spans
        spans = []
        for t in range(NBLK_G):
            s0 = int(off_h[t]) // P
            s1 = int(off_h[t] + gsz_h[t] - 1) // P
            spans.append((s0, s1))

        with tc.tile_pool(name="psh", bufs=6, space="PSUM") as psh, \
                tc.tile_pool(name="ghop", bufs=3) as gpool:
            for k in range(KHOP if _DBG >= 4 else (1 if _DBG >= 2 else 0)):
                xs_src = xs_a if k % 2 == 0 else xs_b
                xs_dst = xs_b if k % 2 == 0 else xs_a
                part = part_a if k % 2 == 0 else part_b
                partv = part_av if k % 2 == 0 else part_bv

                # xs_src DRAM writes (init / previous post-phase) must land
                # before this hop's gathers read them (DRAM is untracked).
                tc.strict_bb_all_engine_barrier()

                tiles = {}
                hi_call = [-1]

                def need_call(s, xs_src=xs_src, tiles=tiles, hi_call=hi_call,
                              toktot=tok_h, idx_t=hop_idx, tabsrc=None, w=D):
                    ci = s // SPC
                    while hi_call[0] < ci:
                        cj = hi_call[0] + 1
                        n_ = min(GCALL, toktot - cj * GCALL)
                        gt = gpool.tile([P, n_ // P, w], bf16, tag="gt")
                        nc.gpsimd.dma_gather(
                            gt[:], xs_src[:],
                            idx_t[:, cj * GCALL // 16:(cj * GCALL + n_) // 16],
                            n_, n_, w, single_packet=False)
                        tiles[cj] = gt
                        tiles.pop(cj - 4, None)
                        hi_call[0] = cj
                    return tiles[ci]

                mcol = 0
                stg = None
                for t in range(NBLK_G):
                    o, l = t // NBLK, t % NBLK
                    s0, s1 = spans[t]
                    acc = psh.tile([P, D], f32, tag="acc")
                    for s in range(s0, s1 + 1):
                        gt = need_call(s)
                        oh = ohp.tile([P, P], bf16, tag="oh")
                        nc.any.tensor_scalar(
                            out=oh[:], in0=iota_b[:],
                            scalar1=hop_loc[:, mcol:mcol + 1],
                            scalar2=None, op0=mybir.AluOpType.is_equal)
                        nc.tensor.matmul(acc[:], lhsT=oh[:],
                                         rhs=gt[:, s % SPC, :],
                                         start=s == s0, stop=s == s1)
                        mcol += 1
                    if l % 16 == 0:
                        stg = stp.tile([P, 16, D], bf16, tag="fst")
                    nc.any.tensor_copy(stg[:, l % 16, :], acc[:])
                    if l % 16 == 15 or l == NBLK - 1:
                        lb = l // 16 * 16
                        nc.sync.dma_start(partv[o, :, lb:l + 1, :],
                                          stg[:, :l - lb + 1, :])
                assert mcol == len(mm_h)

                # reduce-scatter partial -> own shard; update hidden/xs.
                # xs_dst writes are emitted first (they gate the next hop's
                # gathers via the top-of-hop barrier); the hidden updates are
                # appended after and overlap the next hop's compute.
                if _DBG < 3:
                    continue
                rscatter(part[:], rsx[:])
                for l0 in range(0, NBLK, 16):
                    l1 = min(l0 + 16, NBLK)
                    rsb = sb.tile([P, 16, D], bf16, tag="rsb")
                    nc.sync.dma_start(rsb[:, :l1 - l0, :],
                                      xview(rsx)[:, l0:l1, :])
                    if k < KHOP - 1:
                        stg2 = stp.tile([P, 16, D], bf16, tag="xst")
                    for l in range(l0, l1):
                        tmp = sb.tile([P, D], f32, tag="tmp")
                        nc.any.tensor_scalar(out=tmp[:], in0=rsb[:, l - l0, :],
                                             scalar1=bt_sc[:, k, l:l + 1],
                                             scalar2=None,
                                             op0=mybir.AluOpType.mult)
                        nc.any.tensor_tensor(out=hidden[:, l, :],
                                             in0=hidden[:, l, :], in1=tmp[:],
                                             op=mybir.AluOpType.add)
                        if k < KHOP - 1:
                            nc.any.tensor_scalar(out=stg2[:, l - l0, :],
                                                 in0=rsb[:, l - l0, :],
                                                 scalar1=ab_sc[:, l:l + 1],
                                                 scalar2=None,
                                                 op0=mybir.AluOpType.mult)
                    if k < KHOP - 1:
                        nc.sync.dma_start(xview(xs_dst)[:, l0:l1, :],
                                          stg2[:, :l1 - l0, :])

        # ---------- z_ext = [z*e | e | junk] to zs ----------
        zsv = zs.rearrange("(p l) d -> p l d", p=P)
        for l0 in range(0 if _DBG >= 5 else NBLK, NBLK, 8):
            l1 = min(l0 + 8, NBLK)
            zst = stp.tile([P, 8, 2 * D], bf16, tag="zst")
            for l in range(l0, l1):
                prod = sb.tile([P, D], f32, tag="prod")
                nc.any.tensor_tensor(out=prod[:], in0=hidden[:, l, :],
                                     in1=wrep[:], op=mybir.AluOpType.mult)
                scol = sb.tile([P, 1], f32, tag="scol")
                nc.vector.reduce_sum(scol[:], prod[:], axis=mybir.AxisListType.X)
                ecol = sb.tile([P, 1], f32, tag="ecol")
                nc.scalar.activation(ecol[:], scol[:],
                                     mybir.ActivationFunctionType.Exp)
                nc.any.tensor_scalar(out=zst[:, l - l0, 0:D],
                                     in0=hidden[:, l, :], scalar1=ecol[:],
                                     scalar2=None, op0=mybir.AluOpType.mult)
                nc.vector.tensor_copy(zst[:, l - l0, D:D + 1], ecol[:])
                nc.vector.memset(zst[:, l - l0, D + 1:], 0.0)
            nc.sync.dma_start(zsv[:, l0:l1, :], zst[:, :l1 - l0, :])
        # zs writes must land before pool gathers read them
        tc.strict_bb_all_engine_barrier()

        # ---------- pooling ----------
        def pool(tb, idx_t, loc_t, psch):
            gsz_p, tok_p, mm_p, _, _ = psch
            off_p = np.zeros(NSB_G + 1, np.int64)
            off_p[1:] = np.cumsum(gsz_p)
            with tc.tile_pool(name=f"psp{tb}", bufs=4, space="PSUM") as psp, \
                    tc.tile_pool(name=f"gp{tb}", bufs=3) as gpool:
                tiles = {}
                hi_call = [-1]

                def need_call(s):
                    ci = s // SPCP
                    while hi_call[0] < ci:
                        cj = hi_call[0] + 1
                        n_ = min(PCALL, tok_p - cj * PCALL)
                        gt = gpool.tile([P, n_ // P, 2 * D], bf16, tag="gtp")
                        nc.gpsimd.dma_gather(
                            gt[:], zs[:],
                            idx_t[:, cj * PCALL // 16:(cj * PCALL + n_) // 16],
                            n_, n_, 2 * D, single_packet=False)
                        tiles[cj] = gt
                        tiles.pop(cj - 4, None)
                        hi_call[0] = cj
                    return tiles[ci]

                mcol = 0
                stg = None
                for sbk in range(NSB_G):
                    o, l = sbk // NSB, sbk % NSB
                    s0 = int(off_p[sbk]) // P
                    s1 = int(off_p[sbk] + gsz_p[sbk] - 1) // P
                    acc = psp.tile([P, D + 1], f32, tag="pacc")
                    for s in range(s0, s1 + 1):
                        gt = need_call(s)
                        oh = ohp.tile([P, P], bf16, tag="ohp")
                        nc.any.tensor_scalar(
                            out=oh[:], in0=iota_b[:],
                            scalar1=loc_t[:, mcol:mcol + 1],
                            scalar2=None, op0=mybir.AluOpType.is_equal)
                        nc.tensor.matmul(acc[:], lhsT=oh[:],
                                         rhs=gt[:, s % SPCP, 0:D + 1],
                                         start=s == s0, stop=s == s1)
                        mcol += 1
                    if l % 16 == 0:
                        stg = stp.tile([P, 16, D + 2], bf16, tag="pst")
                    nc.any.tensor_copy(stg[:, l % 16, 0:D + 1], acc[:])
                    if l % 16 == 15:
                        lb = l // 16 * 16
                        nc.sync.dma_start(
                            ppv[tb, o, :, lb:l + 1, 0:D + 2], stg[:, :, :])
                assert mcol == len(mm_p)
            if _DBG >= 7:
                tc.strict_bb_all_engine_barrier()
                with tc.tile_critical():
                    ccs_val[0] += 1
                    nc.gpsimd.collective_compute(
                        "ReduceScatter", mybir.AluOpType.add,
                        ins=[pool_part[tb * B:(tb + 1) * B, :]],
                        outs=[rs_p[tb * BSEG:(tb + 1) * BSEG, :]],
                        replica_groups=rg,
                    ).then_inc(ccs, 1)

        if _DBG >= 6:
            pool(0, pool_idx_h, pool_loc_h, psch_h)
            pool(1, pool_idx_t, pool_loc_t, psch_t)
            if _DBG >= 7:
                with tc.tile_critical():
                    nc.gpsimd.wait_ge(ccs, ccs_val[0])
                tc.strict_bb_all_engine_barrier()

        # ---------- normalize + feats + MLP ----------
        with tc.tile_pool(name="psm", bufs=2, space="PSUM") as psm:
            for l in range(NSB if _DBG >= 8 else 1):
                fd = []
                for tb in range(2):
                    raw = sb.tile([P, D + 2], bf16, tag="raw")
                    nc.sync.dma_start(raw[:], rspv[tb, :, l, 0:D + 2])
                    den = sb.tile([P, 1], f32, tag="den")
                    nc.vector.tensor_scalar(out=den[:], in0=raw[:, D:D + 1],
                                            scalar1=1e-30, scalar2=None,
                                            op0=mybir.AluOpType.max)
                    deni = sb.tile([P, 1], f32, tag="deni")
                    nc.vector.reciprocal(deni[:], den[:])
                    pool_sl = sb.tile([P, D], f32, tag="psl")
                    nc.any.tensor_scalar(out=pool_sl[:], in0=raw[:, 0:D],
                                         scalar1=deni[:], scalar2=None,
                                         op0=mybir.AluOpType.mult)
                    pt = psm.tile([P, D], f32, tag="pt")
                    nc.tensor.transpose(out=pt[:], in_=pool_sl[:],
                                        identity=ident[:])
                    fdt = sb.tile([P, D], bf16, tag=f"fd{tb}")
                    nc.any.tensor_copy(fdt[:], pt[:])
                    fd.append(fdt)
                ht = sb.tile([P, D], bf16, tag="fdht")
                nc.any.tensor_tensor(out=ht[:], in0=fd[0][:], in1=fd[1][:],
                                     op=mybir.AluOpType.mult)
                feats = [fd[0], fd[1], ht]

                o1 = sb.tile([P, 4, P], bf16, tag="o1")
                for m in range(4):
                    ps1 = psm.tile([P, P], f32, tag="ps1")
                    for kk in range(3):
                        nc.tensor.matmul(ps1[:],
                                         lhsT=w1t[:, kk, m * P:(m + 1) * P],
                                         rhs=feats[kk][:],
                                         start=kk == 0, stop=kk == 2)
                    nc.scalar.activation(o1[:, m, :], ps1[:],
                                         mybir.ActivationFunctionType.Relu,
                                         bias=b1t[:, m:m + 1])
                ps2 = psm.tile([R, P], f32, tag="ps2")
                for kk in range(4):
                    nc.tensor.matmul(ps2[:], lhsT=w2t[:, kk, :], rhs=o1[:, kk, :],
                                     start=kk == 0, stop=kk == 3)
                lg = sb.tile([R, P], f32, tag="lg")
                nc.vector.tensor_scalar(out=lg[:], in0=ps2[:], scalar1=b2t[:],
                                        scalar2=None, op0=mybir.AluOpType.add)
                lt = psm.tile([P, R], f32, tag="lt")
                nc.tensor.transpose(out=lt[:], in_=lg[:], identity=ident[:R, :R])
                lts = sb.tile([P, R], f32, tag="lts")
                nc.vector.tensor_copy(lts[:], lt[:])
                nc.sync.dma_start(
                    out.rearrange("(l p) r -> p l r", p=P)[:, l, :], lts[:])

    nc.compile()
    return nc


def kernel(embed, temp, attn_w, attn_b, W1, b1, W2, b2,
           edge_index, H_idx, H_seg, T_idx, T_seg, B):
    embed = np.asarray(embed, np.float32)
    temp = np.asarray(temp, np.float32)
    attn_w = np.asarray(attn_w, np.float32)
    W1 = np.asarray(W1, np.float32)
    b1 = np.asarray(b1, np.float32)
    W2 = np.asarray(W2, np.float32)
    b2 = np.asarray(b2, np.float32)
    edge_index = np.asarray(edge_index)
    H_idx, H_seg = np.asarray(H_idx), np.asarray(H_seg)
    T_idx, T_seg = np.asarray(T_idx), np.asarray(T_seg)

    src = edge_index[0].astype(np.int64)
    dst = edge_index[1].astype(np.int64)

    hsch = _prep_hop(src, dst)
    psch_h = _prep_pool(H_idx.astype(np.int64), H_seg.astype(np.int64))
    psch_t = _prep_pool(T_idx.astype(np.int64), T_seg.astype(np.int64))

    key = (hsch[0], psch_h[0], psch_t[0])
    if key not in _COMPILED:
        _COMPILED[key] = _build_program(hsch, psch_h, psch_t)
    nc = _COMPILED[key]

    deg_dst = np.bincount(dst, minlength=NP).astype(np.float64)
    deg_src = np.bincount(src, minlength=NP).astype(np.float64)
    a_full = (1.0 / np.sqrt(np.maximum(deg_src, 1.0))).astype(np.float32)
    b_full = (1.0 / np.sqrt(np.maximum(deg_dst, 1.0))).astype(np.float32)

    emb_full = np.zeros((NP, D), np.float32)
    emb_full[:N] = embed

    bf = ml_dtypes.bfloat16
    in_maps = []
    for c in range(NCORES):
        lo = c * SHARD
        a_c = a_full[lo:lo + SHARD].reshape(NBLK, P).T.copy()
        b_c = b_full[lo:lo + SHARD].reshape(NBLK, P).T.copy()
        bt_c = np.stack([b_c * temp[k + 1] for k in range(3)], axis=1)
        in_maps.append(dict(
            embed_in=np.ascontiguousarray(
                emb_full[lo:lo + SHARD].reshape(NBLK, P, D).transpose(1, 0, 2)),
            temp_in=np.tile(temp[None, :], (P, 1)),
            a_in=a_c,
            ab_in=a_c * b_c,
            bt_in=np.ascontiguousarray(bt_c),
            wrep_in=np.tile(attn_w[:, 0][None, :], (P, 1)),
            w1_in=W1.reshape(3, P, H_MLP).astype(bf),
            b1_in=np.ascontiguousarray(b1.reshape(4, P).T),
            w2_in=W2.reshape(4, P, R).astype(bf),
            b2_in=b2[:, None].copy(),
            hsrc=_wrap_idx16(hsch[3][c]),
            hloc=hsch[4][c],
            psrcH=_wrap_idx16(psch_h[3][c]),
            psrcT=_wrap_idx16(psch_t[3][c]),
            plocH=psch_h[4][c],
            plocT=psch_t[4][c],
        ))

    res = run_bass_kernel_spmd(nc, in_maps, list(range(NCORES)))
    return np.concatenate([res.results[c]["out"] for c in range(NCORES)], axis=0)


# revision 47
# speedup vs baseline: 1.1937x; 1.0429x over previous
import os
import sys

sys.path.insert(0, "/opt/trn_rl_repo")

_DBG = int(os.environ.get("KDBG", "9"))  # phase cutoff for fault bisection

from contextlib import ExitStack

import numpy as np
import ml_dtypes
import concourse.bacc as bacc
import concourse.bass as bass
import concourse.mybir as mybir
from concourse.bass_utils import run_bass_kernel_spmd
from concourse.tile import TileContext
from concourse.library_config import mlp as _mlp_lib
from concourse.masks import make_identity

P = 128
NCORES = 8
N, D, E, KHOP, B, L = 100000, 128, 1600000, 3, 32768, 262144
H_MLP, R = 512, 64
NBLK = 98              # local node blocks per core
SHARD = NBLK * P       # 12544 rows per core
NP = NCORES * SHARD    # 100352 padded rows
NBLK_G = NCORES * NBLK # 784 global dst blocks
NSB_G = B // P         # 256 global segment blocks
BSEG = B // NCORES     # 4096 segments per core
NSB = BSEG // P        # 32 local segment blocks
GCALL = 2048           # tokens per hop dma_gather call
SPC = GCALL // P       # slabs per hop gather call
PCALL = 2048           # tokens per pool dma_gather call (512B rows)
SPCP = PCALL // P      # slabs per pool gather call

f32 = mybir.dt.float32
bf16 = mybir.dt.bfloat16
i16 = mybir.dt.int16
i32 = mybir.dt.int32

_COMPILED = {}


def _wrap_idx16(idx):
    """dma_gather index layout: token i -> partition i%16, col i//16,
    replicated x8 to fill 128 partitions."""
    n = len(idx)
    assert n % 16 == 0
    return np.tile(idx.reshape(n // 16, 16).T.astype(np.int16), (8, 1))


def _grow(local):
    """Gather row of a local node index in the [p, l] table layout."""
    return (local & 127) * NBLK + (local >> 7)


def _schedule(per_core, nb, call):
    """per_core: list of (blk, loc, gidx) arrays, sorted by blk.
    Returns a shared straddled schedule plus per-core token/loc data."""
    cnts = np.zeros((NCORES, nb), np.int64)
    for c, (blk, _, _) in enumerate(per_core):
        cnts[c] = np.bincount(blk, minlength=nb)
    gsz = np.maximum(cnts.max(axis=0), 1)
    off = np.zeros(nb + 1, np.int64)
    off[1:] = np.cumsum(gsz)
    total = int(off[nb])
    tot_pad = (total + call - 1) // call * call
    mm = []  # (block, slab) in consumption order
    for t in range(nb):
        s0, s1 = off[t] // P, (off[t] + gsz[t] - 1) // P
        for s in range(s0, s1 + 1):
            mm.append((t, int(s)))
    M = len(mm)
    gstreams, loccols = [], []
    for c, (blk, loc, gidx) in enumerate(per_core):
        starts = np.concatenate([[0], np.cumsum(cnts[c])])
        pos = off[blk] + (np.arange(len(blk)) - starts[blk])
        g = np.zeros(tot_pad, np.int16)
        g[pos] = gidx
        lstream = np.full(tot_pad, -1.0, np.float32)
        lstream[pos] = loc
        cols = np.full((M, P), -1.0, np.float32)
        ar = np.arange(P)
        for m, (t, s) in enumerate(mm):
            idxs = s * P + ar
            seg = lstream[s * P:(s + 1) * P]
            mask = (idxs >= off[t]) & (idxs < off[t] + gsz[t])
            cols[m] = np.where(mask, seg, -1.0)
        gstreams.append(g)
        loccols.append(np.ascontiguousarray(cols.T))
    return tuple(int(v) for v in gsz), tot_pad, mm, gstreams, loccols


HALF_L = 48  # blocks per owner in RS chunk A (16-aligned); chunk B has 50


def _hop_order():
    ordl = []
    for h in range(2):
        lr = range(0, HALF_L) if h == 0 else range(HALF_L, NBLK)
        for o in range(NCORES):
            for l in lr:
                ordl.append(o * NBLK + l)
    return np.array(ordl, np.int64)


_ORD_H = _hop_order()
_RANK_H = np.empty(NBLK_G, np.int64)
_RANK_H[_ORD_H] = np.arange(NBLK_G)


def _prep_hop(src, dst):
    per_core = []
    for c in range(NCORES):
        m = (src >= c * SHARD) & (src < (c + 1) * SHARD)
        sl = src[m] - c * SHARD
        dg = dst[m]
        t = _RANK_H[dg >> 7]
        order = np.argsort(t, kind="stable")
        t, sl, dg = t[order], sl[order], dg[order]
        per_core.append((t, (dg & 127).astype(np.float32),
                         _grow(sl).astype(np.int16)))
    return _schedule(per_core, NBLK_G, GCALL)


def _prep_pool(idx, seg):
    per_core = []
    for c in range(NCORES):
        m = (idx >= c * SHARD) & (idx < (c + 1) * SHARD)
        r = idx[m] - c * SHARD
        sg = seg[m]
        sb = sg >> 7
        order = np.argsort(sb, kind="stable")
        sb, r, sg = sb[order], r[order], sg[order]
        per_core.append((sb, (sg & 127).astype(np.float32),
                         _grow(r).astype(np.int16)))
    return _schedule(per_core, NSB_G, PCALL)


def _build_program(hsch, psch_h, psch_t):
    nc = bacc.Bacc("TRN2", target_bir_lowering=False, num_devices=NCORES)

    gsz_h, tok_h, mm_h, _, _ = hsch
    gsz_ph, tok_ph, mm_ph, _, _ = psch_h
    gsz_pt, tok_pt, mm_pt, _, _ = psch_t
    off_h = np.zeros(NBLK_G + 1, np.int64)
    off_h[1:] = np.cumsum(gsz_h)

    embed_in = nc.dram_tensor("embed_in", [P, NBLK, D], f32, kind="ExternalInput")
    temp_in = nc.dram_tensor("temp_in", [P, 4], f32, kind="ExternalInput")
    a_in = nc.dram_tensor("a_in", [P, NBLK], f32, kind="ExternalInput")
    ab_in = nc.dram_tensor("ab_in", [P, NBLK], f32, kind="ExternalInput")
    bt_in = nc.dram_tensor("bt_in", [P, 3, NBLK], f32, kind="ExternalInput")
    wrep_in = nc.dram_tensor("wrep_in", [P, D], f32, kind="ExternalInput")
    w1_in = nc.dram_tensor("w1_in", [3, P, H_MLP], bf16, kind="ExternalInput")
    b1_in = nc.dram_tensor("b1_in", [P, 4], f32, kind="ExternalInput")
    w2_in = nc.dram_tensor("w2_in", [4, P, R], bf16, kind="ExternalInput")
    b2_in = nc.dram_tensor("b2_in", [R, 1], f32, kind="ExternalInput")
    hsrc = nc.dram_tensor("hsrc", [P, tok_h // 16], i16, kind="ExternalInput")
    hloc = nc.dram_tensor("hloc", [P, len(mm_h)], f32, kind="ExternalInput")
    psrcH = nc.dram_tensor("psrcH", [P, tok_ph // 16], i16, kind="ExternalInput")
    psrcT = nc.dram_tensor("psrcT", [P, tok_pt // 16], i16, kind="ExternalInput")
    plocH = nc.dram_tensor("plocH", [P, len(mm_ph)], f32, kind="ExternalInput")
    plocT = nc.dram_tensor("plocT", [P, len(mm_pt)], f32, kind="ExternalInput")

    out = nc.dram_tensor("out", [BSEG, R], f32, kind="ExternalOutput")

    xs_a = nc.dram_tensor("xs_a", [SHARD, D], bf16)
    xs_b = nc.dram_tensor("xs_b", [SHARD, D], bf16)
    NHA, NHB = HALF_L, NBLK - HALF_L
    parts = [[nc.dram_tensor(f"part_{s}{h}", [NCORES * P * (NHA if h == 0 else NHB), D], bf16)
              for h in range(2)] for s in range(2)]
    rsx_h = [nc.dram_tensor("rsx_a", [P * NHA, D], bf16),
             nc.dram_tensor("rsx_b", [P * NHB, D], bf16)]
    zs = nc.dram_tensor("zs", [SHARD, 2 * D], bf16)
    pool_part = nc.dram_tensor("pool_part", [2 * B, 2 * D], bf16)
    rs_p = nc.dram_tensor("rs_p", [2 * BSEG, 2 * D], bf16)

    rg = [list(range(NCORES))]

    def xview(t):  # [p, l, d] view of a [SHARD, D] table
        return t.rearrange("(p l) d -> p l d", p=P)

    part_v = [[parts[s][h].rearrange("(o p l) d -> o p l d", o=NCORES, p=P)
               for h in range(2)] for s in range(2)]
    rsx_v = [rsx_h[0].rearrange("(p l) d -> p l d", p=P),
             rsx_h[1].rearrange("(p l) d -> p l d", p=P)]
    ppv = pool_part.rearrange("(t o s l) d -> t o s l d", t=2, o=NCORES, s=P)
    rspv = rs_p.rearrange("(t s l) d -> t s l d", t=2, s=P)

    with TileContext(nc) as tc, ExitStack() as ctx:
        sb = ctx.enter_context(tc.tile_pool(name="sb", bufs=3))
        const = ctx.enter_context(tc.tile_pool(name="const", bufs=1))
        ohp = ctx.enter_context(tc.tile_pool(name="ohp", bufs=6))
        ccs = ctx.enter_context(nc.semaphore("ccs"))
        pfs = ctx.enter_context(nc.semaphore("pfs"))
        ccs_val = [0]
        pfs_val = [0]

        def rscatter(ins_ap, outs_ap):
            tc.strict_bb_all_engine_barrier()
            with tc.tile_critical():
                ccs_val[0] += 1
                nc.gpsimd.collective_compute(
                    "ReduceScatter", mybir.AluOpType.add,
                    ins=[ins_ap], outs=[outs_ap], replica_groups=rg,
                ).then_inc(ccs, 1)
                nc.gpsimd.wait_ge(ccs, ccs_val[0])
            tc.strict_bb_all_engine_barrier()

        nc.gpsimd.load_library(_mlp_lib)

        # ---------- constants ----------
        iota_i = const.tile([P, P], i32)
        nc.gpsimd.iota(iota_i[:], pattern=[[1, P]], base=0, channel_multiplier=0)
        iota_b = const.tile([P, P], bf16)
        nc.vector.tensor_copy(iota_b[:], iota_i[:])
        ident = const.tile([P, P], f32)
        make_identity(nc, ident[:])

        temp_sb = const.tile([P, 4], f32)
        nc.sync.dma_start(temp_sb[:], temp_in[:])
        a_sc = const.tile([P, NBLK], f32)
        nc.sync.dma_start(a_sc[:], a_in[:])
        ab_sc = const.tile([P, NBLK], f32)
        nc.sync.dma_start(ab_sc[:], ab_in[:])
        bt_sc = const.tile([P, 3, NBLK], f32)
        nc.sync.dma_start(bt_sc[:], bt_in[:])
        wrep = const.tile([P, D], f32)
        nc.sync.dma_start(wrep[:], wrep_in[:])
        w1t = const.tile([P, 3, H_MLP], bf16)
        nc.sync.dma_start(w1t[:], w1_in.rearrange("k p h -> p k h")[:])
        b1t = const.tile([P, 4], f32)
        nc.sync.dma_start(b1t[:], b1_in[:])
        w2t = const.tile([P, 4, R], bf16)
        nc.sync.dma_start(w2t[:], w2_in.rearrange("k p r -> p k r")[:])
        b2t = const.tile([R, 1], f32)
        nc.sync.dma_start(b2t[:], b2_in[:])

        hop_idx = const.tile([P, tok_h // 16], i16)
        nc.sync.dma_start(hop_idx[:], hsrc[:])
        hop_loc = const.tile([P, len(mm_h)], f32)
        nc.sync.dma_start(hop_loc[:], hloc[:])
        pool_idx_h = const.tile([P, tok_ph // 16], i16)
        nc.sync.dma_start(pool_idx_h[:], psrcH[:])
        pool_idx_t = const.tile([P, tok_pt // 16], i16)
        nc.sync.dma_start(pool_idx_t[:], psrcT[:])
        pool_loc_h = const.tile([P, len(mm_ph)], f32)
        nc.sync.dma_start(pool_loc_h[:], plocH[:])
        pool_loc_t = const.tile([P, len(mm_pt)], f32)
        nc.sync.dma_start(pool_loc_t[:], plocT[:])

        # ---------- init: hidden = temp0*embed, xs_a = a_sc*embed ----------
        stp = ctx.enter_context(tc.tile_pool(name="stp", bufs=3))
        hidden = const.tile([P, NBLK, D], f32)
        nc.sync.dma_start(hidden[:], embed_in[:])
        for l0 in range(0, NBLK, 16):
            l1 = min(l0 + 16, NBLK)
            stg = stp.tile([P, 16, D], bf16, tag="xst")
            for l in range(l0, l1):
                nc.any.tensor_scalar(out=stg[:, l - l0, :], in0=hidden[:, l, :],
                                     scalar1=a_sc[:, l:l + 1], scalar2=None,
                                     op0=mybir.AluOpType.mult)
            nc.sync.dma_start(xview(xs_a)[:, l0:l1, :], stg[:, :l1 - l0, :])
        nc.vector.tensor_scalar(out=hidden[:], in0=hidden[:],
                                scalar1=temp_sb[:, 0:1], scalar2=None,
                                op0=mybir.AluOpType.mult)

        # ---------- hops ----------
        # precompute per-position slab spans (positions follow _ORD_H)
        spans = []
        for t in range(NBLK_G):
            s0 = int(off_h[t]) // P
            s1 = int(off_h[t] + gsz_h[t] - 1) // P
            spans.append((s0, s1))

        with tc.tile_pool(name="psh", bufs=6, space="PSUM") as psh, \
                tc.tile_pool(name="ghop", bufs=3) as gpool:
            for k in range(KHOP if _DBG >= 4 else (1 if _DBG >= 2 else 0)):
                xs_src = xs_a if k % 2 == 0 else xs_b
                xs_dst = xs_b if k % 2 == 0 else xs_a
                pset = parts[k % 2]
                psetv = part_v[k % 2]

                # xs_src DRAM writes (init / previous post-phase) must land
                # before this hop's gathers read them (DRAM is untracked).
                tc.strict_bb_all_engine_barrier()

                tiles = {}
                hi_call = [-1]

                def need_call(s, xs_src=xs_src, tiles=tiles, hi_call=hi_call,
                              toktot=tok_h, idx_t=hop_idx, tabsrc=None, w=D):
                    ci = s // SPC
                    while hi_call[0] < ci:
                        cj = hi_call[0] + 1
                        n_ = min(GCALL, toktot - cj * GCALL)
                        gt = gpool.tile([P, n_ // P, w], bf16, tag="gt")
                        nc.gpsimd.dma_gather(
                            gt[:], xs_src[:],
                            idx_t[:, cj * GCALL // 16:(cj * GCALL + n_) // 16],
                            n_, n_, w, single_packet=False)
                        tiles[cj] = gt
                        tiles.pop(cj - 4, None)
                        hi_call[0] = cj
                    return tiles[ci]

                mcol = 0
                stg = None
                for pos in range(NBLK_G):
                    t = int(_ORD_H[pos])
                    o, l = t // NBLK, t % NBLK
                    h = 0 if l < HALF_L else 1
                    lh = l - h * HALF_L
                    s0, s1 = spans[pos]
                    acc = psh.tile([P, D], f32, tag="acc")
                    for s in range(s0, s1 + 1):
                        gt = need_call(s)
                        oh = ohp.tile([P, P], bf16, tag="oh")
                        nc.any.tensor_scalar(
                            out=oh[:], in0=iota_b[:],
                            scalar1=hop_loc[:, mcol:mcol + 1],
                            scalar2=None, op0=mybir.AluOpType.is_equal)
                        nc.tensor.matmul(acc[:], lhsT=oh[:],
                                         rhs=gt[:, s % SPC, :],
                                         start=s == s0, stop=s == s1)
                        mcol += 1
                    if lh % 16 == 0:
                        stg = stp.tile([P, 16, D], bf16, tag="fst")
                    nc.any.tensor_copy(stg[:, lh % 16, :], acc[:])
                    nh = NHA if h == 0 else NHB
                    if lh % 16 == 15 or lh == nh - 1:
                        lb = lh // 16 * 16
                        nc.sync.dma_start(psetv[h][o, :, lb:lh + 1, :],
                                          stg[:, :lh - lb + 1, :])
                    if _DBG >= 3 and pos == NCORES * HALF_L - 1:
                        # chunk A complete on all owners: fire its RS while
                        # chunk B is still computing.
                        tc.strict_bb_all_engine_barrier()
                        with tc.tile_critical():
                            ccs_val[0] += 1
                            nc.gpsimd.collective_compute(
                                "ReduceScatter", mybir.AluOpType.add,
                                ins=[pset[0][:]], outs=[rsx_h[0][:]],
                                replica_groups=rg).then_inc(ccs, 1)
                assert mcol == len(mm_h)

                if _DBG < 3:
                    continue
                tc.strict_bb_all_engine_barrier()
                with tc.tile_critical():
                    ccs_val[0] += 1
                    nc.gpsimd.collective_compute(
                        "ReduceScatter", mybir.AluOpType.add,
                        ins=[pset[1][:]], outs=[rsx_h[1][:]],
                        replica_groups=rg).then_inc(ccs, 1)
                    nc.gpsimd.wait_ge(ccs, ccs_val[0])
                tc.strict_bb_all_engine_barrier()
                for l0 in range(0, NBLK, 16):
                    l1 = min(l0 + 16, NBLK)
                    rsb = sb.tile([P, 16, D], bf16, tag="rsb")
                    if l1 <= HALF_L:
                        nc.sync.dma_start(rsb[:, :l1 - l0, :],
                                          rsx_v[0][:, l0:l1, :])
                    else:
                        nc.sync.dma_start(
                            rsb[:, :l1 - l0, :],
                            rsx_v[1][:, l0 - HALF_L:l1 - HALF_L, :])
                    if k < KHOP - 1:
                        stg2 = stp.tile([P, 16, D], bf16, tag="xst")
                    for l in range(l0, l1):
                        tmp = sb.tile([P, D], f32, tag="tmp")
                        nc.any.tensor_scalar(out=tmp[:], in0=rsb[:, l - l0, :],
                                             scalar1=bt_sc[:, k, l:l + 1],
                                             scalar2=None,
                                             op0=mybir.AluOpType.mult)
                        nc.any.tensor_tensor(out=hidden[:, l, :],
                                             in0=hidden[:, l, :], in1=tmp[:],
                                             op=mybir.AluOpType.add)
                        if k < KHOP - 1:
                            nc.any.tensor_scalar(out=stg2[:, l - l0, :],
                                                 in0=rsb[:, l - l0, :],
                                                 scalar1=ab_sc[:, l:l + 1],
                                                 scalar2=None,
                                                 op0=mybir.AluOpType.mult)
                    if k < KHOP - 1:
                        nc.sync.dma_start(xview(xs_dst)[:, l0:l1, :],
                                          stg2[:, :l1 - l0, :])

        # ---------- z_ext = [z*e | e | junk] to zs ----------
        zsv = zs.rearrange("(p l) d -> p l d", p=P)
        for l0 in range(0 if _DBG >= 5 else NBLK, NBLK, 8):
            l1 = min(l0 + 8, NBLK)
            zst = stp.tile([P, 8, 2 * D], bf16, tag="zst")
            for l in range(l0, l1):
                prod = sb.tile([P, D], f32, tag="prod")
                nc.any.tensor_tensor(out=prod[:], in0=hidden[:, l, :],
                                     in1=wrep[:], op=mybir.AluOpType.mult)
                scol = sb.tile([P, 1], f32, tag="scol")
                nc.vector.reduce_sum(scol[:], prod[:], axis=mybir.AxisListType.X)
                ecol = sb.tile([P, 1], f32, tag="ecol")
                nc.scalar.activation(ecol[:], scol[:],
                                     mybir.ActivationFunctionType.Exp)
                nc.any.tensor_scalar(out=zst[:, l - l0, 0:D],
                                     in0=hidden[:, l, :], scalar1=ecol[:],
                                     scalar2=None, op0=mybir.AluOpType.mult)
                nc.vector.tensor_copy(zst[:, l - l0, D:D + 1], ecol[:])
                nc.vector.memset(zst[:, l - l0, D + 1:], 0.0)
            nc.sync.dma_start(zsv[:, l0:l1, :], zst[:, :l1 - l0, :])
        # zs writes must land before pool gathers read them
        tc.strict_bb_all_engine_barrier()

        # ---------- pooling ----------
        def pool(tb, idx_t, loc_t, psch):
            gsz_p, tok_p, mm_p, _, _ = psch
            off_p = np.zeros(NSB_G + 1, np.int64)
            off_p[1:] = np.cumsum(gsz_p)
            with tc.tile_pool(name=f"psp{tb}", bufs=4, space="PSUM") as psp, \
                    tc.tile_pool(name=f"gp{tb}", bufs=3) as gpool:
                tiles = {}
                hi_call = [-1]

                def need_call(s):
                    ci = s // SPCP
                    while hi_call[0] < ci:
                        cj = hi_call[0] + 1
                        n_ = min(PCALL, tok_p - cj * PCALL)
                        gt = gpool.tile([P, n_ // P, 2 * D], bf16, tag="gtp")
                        nc.gpsimd.dma_gather(
                            gt[:], zs[:],
                            idx_t[:, cj * PCALL // 16:(cj * PCALL + n_) // 16],
                            n_, n_, 2 * D, single_packet=False)
                        tiles[cj] = gt
                        tiles.pop(cj - 4, None)
                        hi_call[0] = cj
                    return tiles[ci]

                mcol = 0
                stg = None
                for sbk in range(NSB_G):
                    o, l = sbk // NSB, sbk % NSB
                    s0 = int(off_p[sbk]) // P
                    s1 = int(off_p[sbk] + gsz_p[sbk] - 1) // P
                    acc = psp.tile([P, D + 1], f32, tag="pacc")
                    for s in range(s0, s1 + 1):
                        gt = need_call(s)
                        oh = ohp.tile([P, P], bf16, tag="ohp")
                        nc.any.tensor_scalar(
                            out=oh[:], in0=iota_b[:],
                            scalar1=loc_t[:, mcol:mcol + 1],
                            scalar2=None, op0=mybir.AluOpType.is_equal)
                        nc.tensor.matmul(acc[:], lhsT=oh[:],
                                         rhs=gt[:, s % SPCP, 0:D + 1],
                                         start=s == s0, stop=s == s1)
                        mcol += 1
                    if l % 16 == 0:
                        stg = stp.tile([P, 16, D + 2], bf16, tag="pst")
                    nc.any.tensor_copy(stg[:, l % 16, 0:D + 1], acc[:])
                    if l % 16 == 15:
                        lb = l // 16 * 16
                        nc.sync.dma_start(
                            ppv[tb, o, :, lb:l + 1, 0:D + 2], stg[:, :, :])
                assert mcol == len(mm_p)
            if _DBG >= 7:
                tc.strict_bb_all_engine_barrier()
                with tc.tile_critical():
                    ccs_val[0] += 1
                    nc.gpsimd.collective_compute(
                        "ReduceScatter", mybir.AluOpType.add,
                        ins=[pool_part[tb * B:(tb + 1) * B, :]],
                        outs=[rs_p[tb * BSEG:(tb + 1) * BSEG, :]],
                        replica_groups=rg,
                    ).then_inc(ccs, 1)

        if _DBG >= 6:
            pool(0, pool_idx_h, pool_loc_h, psch_h)
            pool(1, pool_idx_t, pool_loc_t, psch_t)
            if _DBG >= 7:
                with tc.tile_critical():
                    nc.gpsimd.wait_ge(ccs, ccs_val[0])
                tc.strict_bb_all_engine_barrier()

        # ---------- normalize + feats + MLP ----------
        with tc.tile_pool(name="psm", bufs=2, space="PSUM") as psm:
            for l in range(NSB if _DBG >= 8 else 1):
                fd = []
                for tb in range(2):
                    raw = sb.tile([P, D + 2], bf16, tag="raw")
                    nc.sync.dma_start(raw[:], rspv[tb, :, l, 0:D + 2])
                    den = sb.tile([P, 1], f32, tag="den")
                    nc.vector.tensor_scalar(out=den[:], in0=raw[:, D:D + 1],
                                            scalar1=1e-30, scalar2=None,
                                            op0=mybir.AluOpType.max)
                    deni = sb.tile([P, 1], f32, tag="deni")
                    nc.vector.reciprocal(deni[:], den[:])
                    pool_sl = sb.tile([P, D], f32, tag="psl")
                    nc.any.tensor_scalar(out=pool_sl[:], in0=raw[:, 0:D],
                                         scalar1=deni[:], scalar2=None,
                                         op0=mybir.AluOpType.mult)
                    pt = psm.tile([P, D], f32, tag="pt")
                    nc.tensor.transpose(out=pt[:], in_=pool_sl[:],
                                        identity=ident[:])
                    fdt = sb.tile([P, D], bf16, tag=f"fd{tb}")
                    nc.any.tensor_copy(fdt[:], pt[:])
                    fd.append(fdt)
                ht = sb.tile([P, D], bf16, tag="fdht")
                nc.any.tensor_tensor(out=ht[:], in0=fd[0][:], in1=fd[1][:],
                                     op=mybir.AluOpType.mult)
                feats = [fd[0], fd[1], ht]

                o1 = sb.tile([P, 4, P], bf16, tag="o1")
                for m in range(4):
                    ps1 = psm.tile([P, P], f32, tag="ps1")
                    for kk in range(3):
                        nc.tensor.matmul(ps1[:],
                                         lhsT=w1t[:, kk, m * P:(m + 1) * P],
                                         rhs=feats[kk][:],
                                         start=kk == 0, stop=kk == 2)
                    nc.scalar.activation(o1[:, m, :], ps1[:],
                                         mybir.ActivationFunctionType.Relu,
                                         bias=b1t[:, m:m + 1])
                ps2 = psm.tile([R, P], f32, tag="ps2")
                for kk in range(4):
                    nc.tensor.matmul(ps2[:], lhsT=w2t[:, kk, :], rhs=o1[:, kk, :],
                                     start=kk == 0, stop=kk == 3)
                lg = sb.tile([R, P], f32, tag="lg")
                nc.vector.tensor_scalar(out=lg[:], in0=ps2[:], scalar1=b2t[:],
                                        scalar2=None, op0=mybir.AluOpType.add)
                lt = psm.tile([P, R], f32, tag="lt")
                nc.tensor.transpose(out=lt[:], in_=lg[:], identity=ident[:R, :R])
                lts = sb.tile([P, R], f32, tag="lts")
                nc.vector.tensor_copy(lts[:], lt[:])
                nc.sync.dma_start(
                    out.rearrange("(l p) r -> p l r", p=P)[:, l, :], lts[:])

    nc.compile()
    return nc


def kernel(embed, temp, attn_w, attn_b, W1, b1, W2, b2,
           edge_index, H_idx, H_seg, T_idx, T_seg, B):
    embed = np.asarray(embed, np.float32)
    temp = np.asarray(temp, np.float32)
    attn_w = np.asarray(attn_w, np.float32)
    W1 = np.asarray(W1, np.float32)
    b1 = np.asarray(b1, np.float32)
    W2 = np.asarray(W2, np.float32)
    b2 = np.asarray(b2, np.float32)
    edge_index = np.asarray(edge_index)
    H_idx, H_seg = np.asarray(H_idx), np.asarray(H_seg)
    T_idx, T_seg = np.asarray(T_idx), np.asarray(T_seg)

    src = edge_index[0].astype(np.int64)
    dst = edge_index[1].astype(np.int64)

    hsch = _prep_hop(src, dst)
    psch_h = _prep_pool(H_idx.astype(np.int64), H_seg.astype(np.int64))
    psch_t = _prep_pool(T_idx.astype(np.int64), T_seg.astype(np.int64))

    key = (hsch[0], psch_h[0], psch_t[0])
    if key not in _COMPILED:
        _COMPILED[key] = _build_program(hsch, psch_h, psch_t)
    nc = _COMPILED[key]

    deg_dst = np.bincount(dst, minlength=NP).astype(np.float64)
    deg_src = np.bincount(src, minlength=NP).astype(np.float64)
    a_full = (1.0 / np.sqrt(np.maximum(deg_src, 1.0))).astype(np.float32)
    b_full = (1.0 / np.sqrt(np.maximum(deg_dst, 1.0))).astype(np.float32)

    emb_full = np.zeros((NP, D), np.float32)
    emb_full[:N] = embed

    bf = ml_dtypes.bfloat16
    in_maps = []
    for c in range(NCORES):
        lo = c * SHARD
        a_c = a_full[lo:lo + SHARD].reshape(NBLK, P).T.copy()
        b_c = b_full[lo:lo + SHARD].reshape(NBLK, P).T.copy()
        bt_c = np.stack([b_c * temp[k + 1] for k in range(3)], axis=1)
        in_maps.append(dict(
            embed_in=np.ascontiguousarray(
                emb_full[lo:lo + SHARD].reshape(NBLK, P, D).transpose(1, 0, 2)),
            temp_in=np.tile(temp[None, :], (P, 1)),
            a_in=a_c,
            ab_in=a_c * b_c,
            bt_in=np.ascontiguousarray(bt_c),
            wrep_in=np.tile(attn_w[:, 0][None, :], (P, 1)),
            w1_in=W1.reshape(3, P, H_MLP).astype(bf),
            b1_in=np.ascontiguousarray(b1.reshape(4, P).T),
            w2_in=W2.reshape(4, P, R).astype(bf),
            b2_in=b2[:, None].copy(),
            hsrc=_wrap_idx16(hsch[3][c]),
            hloc=hsch[4][c],
            psrcH=_wrap_idx16(psch_h[3][c]),
            psrcT=_wrap_idx16(psch_t[3][c]),
            plocH=psch_h[4][c],
            plocT=psch_t[4][c],
        ))

    res = run_bass_kernel_spmd(nc, in_maps, list(range(NCORES)))
    return np.concatenate([res.results[c]["out"] for c in range(NCORES)], axis=0)
